# revision 12
# baseline (speedup 1.0000x reference)
"""Brute-force kNN graph (PyG knn_graph style) on 8 Trainium2 NeuronCores.

Strategy (data-parallel, row-sharded):
  - Each core owns 2500 query rows of the 20000x20000 distance matrix.
  - Device computes v_ij = 2*q_i.p_j - |p_j|^2 (row-shift of -d2, so per-row
    ranking equals nearest-neighbor ranking) via a K=14 split-fp16 matmul
    (fp16 hi/lo decomposition -> exact products accumulated in f32 PSUM,
    1 cycle/column on the PE vs 4 for fp32).
  - VectorE max (top-8) + max_index per 2048-column chunk -> 80 candidate
    indices per row (uint16, chunk-local).
  - Host rescores the 80 candidates exactly (same f32 Gram-trick ops as the
    reference), sorts by (d2, index) to match jax.lax.top_k tie-breaking,
    drops self, takes k. A saturated-chunk safety check (a chunk whose all 8
    returned candidates fall below the row's 18th-best + margin could hide
    more neighbors) routes rare rows to an exact full numpy rescan.
"""

import os
import sys

import numpy as np

for _p in ("/opt/trn_rl_repo", "/root/.axon_site/_ro/trn_rl_repo"):
    if os.path.isdir(_p) and _p not in sys.path:
        sys.path.append(_p)

from concourse import bass, mybir  # noqa: E402
from concourse.bass_utils import run_bass_kernel_spmd  # noqa: E402
from concourse.tile import TileContext  # noqa: E402

N = 20000
KNN = 17
NCORES = 8
R = N // NCORES          # 2500 query rows per core
CHUNK = 2048             # candidate-chunk width (4 PSUM banks)
NCHUNK = 10
NPAD = CHUNK * NCHUNK    # 20480 columns (480 padded "far" points)
BLOCK = 125              # query rows per row-block (20 uniform blocks)
NBLOCK = R // BLOCK
KDIM = 14                # split-fp16 contraction depth
MARGIN = 0.1             # d2-units slack for the saturation check

_last_exec_time_ns = None
_last_results = None


def build_kernel(r=R, npad=NPAD, chunk=CHUNK, block=BLOCK, kdim=KDIM):
    """One-core Bass graph (SPMD across all cores).

    Inputs : qT [kdim, r] fp16 (stationary rows for this core's queries)
             pT [kdim, npad] fp16 (moving rows for all points)
    Output : cand [r, nchunk*8] uint16 (chunk-local top-8 indices)
    """
    nchunk = npad // chunk
    nblock = r // block
    assert r % block == 0 and npad % chunk == 0 and chunk % 512 == 0

    nc = bass.Bass()
    f16, f32, u16 = mybir.dt.float16, mybir.dt.float32, mybir.dt.uint16
    # Single combined input => one DMA transfer => one DMA semaphore for
    # consumers (walrus rejects instructions waiting on multiple DMA sems).
    qpT = nc.declare_dram_parameter("qpT", [kdim, r + npad], f16, isOutput=False)
    # Output stays in SBUF-native layout [block, nblock*nchunk*8]: row
    # (b*block+p)'s candidates live at cand[p, b*nchunk*8 : (b+1)*nchunk*8].
    # One contiguous DMA at the end; the host undoes the interleave.
    cand = nc.declare_dram_parameter(
        "cand", [block, nblock * nchunk * 8], u16, isOutput=True
    )

    with TileContext(nc) as tc:
        with (
            tc.tile_pool(name="const", bufs=1) as cpool,
            tc.tile_pool(name="psum", bufs=2, space="PSUM") as ppool,
            tc.tile_pool(name="outs", bufs=3) as opool,
            tc.tile_pool(name="mx", bufs=4) as mpool,
        ):
            qp_sb = cpool.tile([kdim, r + npad], f16)
            nc.sync.dma_start(out=qp_sb, in_=qpT[:, :])
            qT_sb = qp_sb[:, :r]
            pT_sb = qp_sb[:, r:]

            out_sb = opool.tile([block, nblock * nchunk * 8], u16)
            for b in range(nblock):
                lhsT = qT_sb[:, b * block : (b + 1) * block]
                for ch in range(nchunk):
                    ps = ppool.tile([block, chunk], f32)
                    for s in range(chunk // 512):
                        col = ch * chunk + s * 512
                        nc.tensor.matmul(
                            ps[:, s * 512 : (s + 1) * 512],
                            lhsT=lhsT,
                            rhs=pT_sb[:, col : col + 512],
                            start=True,
                            stop=True,
                        )
                    vmax = mpool.tile([block, 8], f32)
                    nc.vector.max(out=vmax, in_=ps)
                    nc.vector.max_index(
                        out=out_sb[:, (b * nchunk + ch) * 8 : (b * nchunk + ch + 1) * 8],
                        in_max=vmax,
                        in_values=ps,
                    )
            nc.sync.dma_start(out=cand[:, :], in_=out_sb)

    _elide_redundant_waits(nc)
    _fix_tail_drain(nc)
    return nc


def _fix_tail_drain(nc):
    """Walrus allows ~1 sync-wait per instruction; Tile's kernel-tail global
    drain carries one wait per logical proc. Engine/sequencer-completion
    waits are subsumed by the all-engine barrier that follows (each engine's
    own drain implies its queue is empty and its sem updates applied), so
    drop them. Spread the remaining DMA-queue waits one-per-instruction
    across the global drain and the barrier's per-engine drains (whose own
    `sem >= 0` waits are vacuous). Must not touch anything after the
    barrier's semaphore reset, so only the first barrier's drains are used.
    """
    import re

    insts = []
    for blk in nc.m.functions[0].blocks:
        insts.extend(blk.instructions)

    gi = None
    for k, inst in enumerate(insts):
        si = inst.sync_info
        if type(inst).__name__ == "InstDrain" and si and si.on_wait and len(si.on_wait) > 1:
            gi = k
    assert gi is not None, "global tail drain not found"
    drain = insts[gi]

    dma_waits = []
    for w in drain.sync_info.on_wait:
        name = w.ant_name or ""
        if re.fullmatch(r"DMA(HW|SW)\d+_\d+", name):
            dma_waits.append(w)
        # engine/sequencer completion waits: dropped (barrier-subsumed)
    drain.sync_info.on_wait = dma_waits[:1]
    rest = dma_waits[1:]

    k = gi + 1
    while rest and k < len(insts):
        inst = insts[k]
        k += 1
        if type(inst).__name__ != "InstDrain":
            continue
        si = inst.sync_info
        if si is None or (si.on_wait and any(w.wait_value for w in si.on_wait)):
            continue  # only reuse drains whose waits are vacuous (>= 0)
        si.on_wait = [rest.pop(0)]
    assert not rest, f"{len(rest)} tail DMA waits left unplaced"


def _elide_redundant_waits(nc):
    """Drop sync waits that are redundant under per-engine program order.

    Walrus rejects compute instructions carrying >1 sync-wait command. Tile
    emits (a) waits on the instruction's own engine-completion semaphore
    (engines execute their queue in order, so these always hold by issue
    time), and (b) waits already dominated by an equal-or-higher wait on the
    same semaphore earlier in the same engine's queue (queue heads block on
    waits, so later instructions inherit them). Both classes are removable.
    """
    import re

    enforced = {}  # engine name -> {sem name: max waited value}
    for blk in nc.m.functions[0].blocks:
        for inst in blk.instructions:
            si = inst.sync_info
            if si is None or not si.on_wait:
                continue
            if type(inst).__name__ in ("InstDrain", "InstEventSemaphore"):
                continue
            eng = inst.engine.name if inst.engine is not None else None
            if eng is None:
                continue
            is_dma = type(inst).__name__ == "InstDMACopy"
            seen = enforced.setdefault(eng, {})
            keep = []
            for w in si.on_wait:
                name = w.ant_name or ""
                val = w.wait_value
                if re.fullmatch(rf"{re.escape(eng)}_\d+", name):
                    continue  # own-engine completion sem
                if is_dma and re.fullmatch(r"DMA(HW|SW)\d+_\d+", name):
                    continue  # same-queue DMA ordering is in-order by HW
                if val is not None and seen.get(name, -1) >= val:
                    continue  # dominated by an earlier wait on this queue
                keep.append(w)
                if val is not None:
                    seen[name] = max(seen.get(name, -1), val)
            if len(keep) != len(si.on_wait):
                si.on_wait = keep


def _split_f16(a32):
    """fp32 array -> (hi, lo) fp16 pair with hi+lo ~ a to ~22 bits."""
    hi = a32.astype(np.float16)
    lo = (a32 - hi.astype(np.float32)).astype(np.float16)
    return hi, lo


def _prep_operands(pos32):
    """Build qT_all [14, N] and pT [14, NPAD] fp16 operand stacks.

    Row pairing (lhs row, rhs row) so that sum_k lhs[k,i]*rhs[k,j] =
    2*q_i.p_j - x2_j  with fp16 hi/lo products exact in f32:
      per coord c: (2qh, ph), (2qh, pl), (2ql, ph), (2ql, pl)   -> 12 rows
      (1, -x2h), (1, -x2l)                                      -> 2 rows
    """
    ph, pl = _split_f16(pos32)                      # [N,3] each
    x2_64 = np.sum(pos32.astype(np.float64) ** 2, axis=-1)
    x2h = x2_64.astype(np.float32).astype(np.float16)
    x2l = (x2_64 - x2h.astype(np.float64)).astype(np.float32).astype(np.float16)

    qT = np.zeros((KDIM, N), np.float16)
    pTf = np.zeros((KDIM, NPAD), np.float16)
    row = 0
    for c in range(3):
        qh2 = (2.0 * ph[:, c].astype(np.float32)).astype(np.float16)
        ql2 = (2.0 * pl[:, c].astype(np.float32)).astype(np.float16)
        for qrow, prow in ((qh2, ph[:, c]), (qh2, pl[:, c]),
                           (ql2, ph[:, c]), (ql2, pl[:, c])):
            qT[row] = qrow
            pTf[row, :N] = prow
            row += 1
    qT[row] = np.float16(1.0)
    pTf[row, :N] = -x2h
    pTf[row, N:] = np.float16(-60000.0)  # padded columns always lose
    row += 1
    qT[row] = np.float16(1.0)
    pTf[row, :N] = -x2l
    row += 1
    assert row == KDIM
    return qT, pTf


def _postprocess(pos32, cand_all, k):
    """cand_all [N, NCHUNK*8] uint16 chunk-local indices -> (edge_index, dist)."""
    n = pos32.shape[0]
    x2 = np.sum(pos32 * pos32, axis=-1)  # f32, same op order as reference

    base = (np.arange(NCHUNK * 8, dtype=np.int32) // 8) * CHUNK
    gj = cand_all.astype(np.int32) + base[None, :]          # global ids
    valid = gj < n
    gjc = np.minimum(gj, n - 1)

    pj = pos32[gjc]                                          # [n, 80, 3]
    dot = np.einsum("nd,ncd->nc", pos32, pj)
    d2r = x2[:, None] + x2[gjc] - 2.0 * dot                  # rescored, f32
    d2r = np.where(valid, d2r, np.inf).astype(np.float32)

    rows = np.arange(n, dtype=np.int32)[:, None]
    d2m = np.where(gj == rows, np.inf, d2r)                  # self excluded

    order = np.lexsort((gjc, d2m), axis=-1)
    sel = order[:, :k]
    selj = np.take_along_axis(gj, sel, 1)
    seld2 = np.take_along_axis(d2m, sel, 1)

    # --- safety: rows where a saturated chunk could hide a missed neighbor
    part = np.partition(d2r, k, axis=1)                      # self included
    cutoff = part[:, k]                                      # (k+1)-th smallest
    chunk_max = d2r.reshape(n, NCHUNK, 8).max(axis=2)
    flagged = np.any(chunk_max <= cutoff[:, None] + MARGIN, axis=1)
    flagged |= ~np.isfinite(seld2[:, -1])

    for i in np.nonzero(flagged)[0]:
        d2i = x2[i] + x2 - 2.0 * (pos32 @ pos32[i])
        d2i[i] = np.inf
        oi = np.lexsort((np.arange(n), d2i))[:k]
        selj[i] = oi
        seld2[i] = d2i[oi]

    dist = np.sqrt(np.maximum(seld2, 0.0)).astype(np.float32)
    dst = np.repeat(np.arange(n, dtype=np.int32), k)
    edge_index = np.stack([selj.reshape(-1).astype(np.int32), dst])
    return edge_index, dist.reshape(-1)


def kernel(pos, k):
    global _last_exec_time_ns, _last_results
    k = int(k)
    pos32 = np.ascontiguousarray(np.asarray(pos), dtype=np.float32)
    assert pos32.shape == (N, 3), pos32.shape

    qT_all, pTf = _prep_operands(pos32)
    in_maps = [
        {
            "qpT": np.ascontiguousarray(
                np.concatenate([qT_all[:, c * R : (c + 1) * R], pTf], axis=1)
            )
        }
        for c in range(NCORES)
    ]

    nc = build_kernel()
    res = run_bass_kernel_spmd(nc, in_maps, core_ids=list(range(NCORES)))
    _last_exec_time_ns = res.exec_time_ns
    _last_results = res

    def _decode(arr):
        # [block, nblock*nchunk*8] -> [r, nchunk*8] (row b*block+p = arr[p, b])
        a = arr.reshape(BLOCK, R // BLOCK, NCHUNK * 8)
        return a.transpose(1, 0, 2).reshape(R, NCHUNK * 8)

    cand_all = np.concatenate(
        [_decode(res.results[c]["cand"]) for c in range(NCORES)], axis=0
    )
    assert cand_all.shape == (N, NCHUNK * 8)
    return _postprocess(pos32, cand_all, k)


# revision 22
# speedup vs baseline: 1.3190x; 1.3190x over previous
"""Brute-force kNN graph (PyG knn_graph style) on 8 Trainium2 NeuronCores.

Strategy (data-parallel, row-sharded):
  - Each core owns 2500 query rows of the 20000x20000 distance matrix.
  - Device computes v_ij = 2*q_i.p_j - |p_j|^2 (row-shift of -d2, so per-row
    ranking equals nearest-neighbor ranking) via a K=14 split-fp16 matmul
    (fp16 hi/lo decomposition -> exact products accumulated in f32 PSUM,
    1 cycle/column on the PE vs 4 for fp32).
  - VectorE max (top-8) + max_index per 2048-column chunk -> 80 candidate
    indices per row (uint16, chunk-local).
  - Host rescores the 80 candidates exactly (same f32 Gram-trick ops as the
    reference), sorts by (d2, index) to match jax.lax.top_k tie-breaking,
    drops self, takes k. A saturated-chunk safety check (a chunk whose all 8
    returned candidates fall below the row's 18th-best + margin could hide
    more neighbors) routes rare rows to an exact full numpy rescan.
"""

import os
import sys

import numpy as np

for _p in ("/opt/trn_rl_repo", "/root/.axon_site/_ro/trn_rl_repo"):
    if os.path.isdir(_p) and _p not in sys.path:
        sys.path.append(_p)

from concourse import bass, mybir  # noqa: E402
from concourse.bass_utils import run_bass_kernel_spmd  # noqa: E402
from concourse.tile import TileContext  # noqa: E402
from concourse.tile_rust import add_dep_helper  # noqa: E402

N = 20000
KNN = 17
NCORES = 8
R = N // NCORES          # 2500 query rows per core
CHUNK = 2048             # candidate-chunk width (4 PSUM banks)
NCHUNK = 10
NPAD = CHUNK * NCHUNK    # 20480 columns (480 padded "far" points)
BLOCK = 125              # query rows per row-block (20 uniform blocks)
NBLOCK = R // BLOCK
KDIM = 14                # split-fp16 contraction depth
MARGIN = 0.1             # d2-units slack for the saturation check

_last_exec_time_ns = None
_last_results = None


def build_kernel(r=R, npad=NPAD, chunk=CHUNK, block=BLOCK, kdim=KDIM):
    """One-core Bass graph (SPMD across all cores).

    Inputs : qT [kdim, r] fp16 (stationary rows for this core's queries)
             pT [kdim, npad] fp16 (moving rows for all points)
    Output : cand [r, nchunk*8] uint16 (chunk-local top-8 indices)
    """
    nchunk = npad // chunk
    nblock = r // block
    assert r % block == 0 and npad % chunk == 0 and chunk % 512 == 0

    nc = bass.Bass()
    f16, f32, u16 = mybir.dt.float16, mybir.dt.float32, mybir.dt.uint16
    # Single combined input => one DMA transfer => one DMA semaphore for
    # consumers (walrus rejects instructions waiting on multiple DMA sems).
    qpT = nc.declare_dram_parameter("qpT", [kdim, r + npad], f16, isOutput=False)
    # Output stays in SBUF-native layout [block, nblock*nchunk*8]: row
    # (b*block+p)'s candidates live at cand[p, b*nchunk*8 : (b+1)*nchunk*8].
    # One contiguous DMA at the end; the host undoes the interleave.
    cand = nc.declare_dram_parameter(
        "cand", [block, nblock * nchunk * 8], u16, isOutput=True
    )

    with TileContext(nc) as tc:
        with (
            tc.tile_pool(name="const", bufs=1) as cpool,
            tc.tile_pool(name="psum", bufs=2, space="PSUM") as ppool,
            tc.tile_pool(name="outs", bufs=3) as opool,
            tc.tile_pool(name="mx", bufs=4) as mpool,
        ):
            qp_sb = cpool.tile([kdim, r + npad], f16)
            nc.sync.dma_start(out=qp_sb, in_=qpT[:, :])
            qT_sb = qp_sb[:, :r]
            pT_sb = qp_sb[:, r:]

            out_sb = opool.tile([block, nblock * nchunk * 8], u16)
            for b in range(nblock):
                lhsT = qT_sb[:, b * block : (b + 1) * block]
                for ch in range(nchunk):
                    ps = ppool.tile([block, chunk], f32)
                    for s in range(chunk // 512):
                        col = ch * chunk + s * 512
                        nc.tensor.matmul(
                            ps[:, s * 512 : (s + 1) * 512],
                            lhsT=lhsT,
                            rhs=pT_sb[:, col : col + 512],
                            start=True,
                            stop=True,
                        )
                    vmax = mpool.tile([block, 8], f32)
                    nc.vector.max(out=vmax, in_=ps)
                    nc.vector.max_index(
                        out=out_sb[:, (b * nchunk + ch) * 8 : (b * nchunk + ch + 1) * 8],
                        in_max=vmax,
                        in_values=ps,
                    )
            nc.sync.dma_start(out=cand[:, :], in_=out_sb)

    _elide_redundant_waits(nc)
    _fix_tail_drain(nc)
    return nc


def _fix_tail_drain(nc):
    """Walrus allows ~1 sync-wait per instruction; Tile's kernel-tail global
    drain carries one wait per logical proc. Engine/sequencer-completion
    waits are subsumed by the all-engine barrier that follows (each engine's
    own drain implies its queue is empty and its sem updates applied), so
    drop them. Spread the remaining DMA-queue waits one-per-instruction
    across the global drain and the barrier's per-engine drains (whose own
    `sem >= 0` waits are vacuous). Must not touch anything after the
    barrier's semaphore reset, so only the first barrier's drains are used.
    """
    import re

    insts = []
    for blk in nc.m.functions[0].blocks:
        insts.extend(blk.instructions)

    gi = None
    for k, inst in enumerate(insts):
        si = inst.sync_info
        if type(inst).__name__ == "InstDrain" and si and si.on_wait and len(si.on_wait) > 1:
            gi = k
    assert gi is not None, "global tail drain not found"
    drain = insts[gi]

    dma_waits = []
    for w in drain.sync_info.on_wait:
        name = w.ant_name or ""
        if re.fullmatch(r"DMA(HW|SW)\d+_\d+", name):
            dma_waits.append(w)
        # engine/sequencer completion waits: dropped (barrier-subsumed)
    drain.sync_info.on_wait = dma_waits[:1]
    rest = dma_waits[1:]

    k = gi + 1
    while rest and k < len(insts):
        inst = insts[k]
        k += 1
        if type(inst).__name__ != "InstDrain":
            continue
        si = inst.sync_info
        if si is None or (si.on_wait and any(w.wait_value for w in si.on_wait)):
            continue  # only reuse drains whose waits are vacuous (>= 0)
        si.on_wait = [rest.pop(0)]
    assert not rest, f"{len(rest)} tail DMA waits left unplaced"


def _elide_redundant_waits(nc):
    """Drop sync waits that are redundant under per-engine program order.

    Walrus rejects compute instructions carrying >1 sync-wait command. Tile
    emits (a) waits on the instruction's own engine-completion semaphore
    (engines execute their queue in order, so these always hold by issue
    time), and (b) waits already dominated by an equal-or-higher wait on the
    same semaphore earlier in the same engine's queue (queue heads block on
    waits, so later instructions inherit them). Both classes are removable.
    """
    import re

    enforced = {}  # engine name -> {sem name: max waited value}
    for blk in nc.m.functions[0].blocks:
        for inst in blk.instructions:
            si = inst.sync_info
            if si is None or not si.on_wait:
                continue
            if type(inst).__name__ in ("InstDrain", "InstEventSemaphore"):
                continue
            eng = inst.engine.name if inst.engine is not None else None
            if eng is None:
                continue
            is_dma = type(inst).__name__ == "InstDMACopy"
            seen = enforced.setdefault(eng, {})
            keep = []
            for w in si.on_wait:
                name = w.ant_name or ""
                val = w.wait_value
                if re.fullmatch(rf"{re.escape(eng)}_\d+", name):
                    continue  # own-engine completion sem
                if is_dma and re.fullmatch(r"DMA(HW|SW)\d+_\d+", name):
                    continue  # same-queue DMA ordering is in-order by HW
                if val is not None and seen.get(name, -1) >= val:
                    continue  # dominated by an earlier wait on this queue
                keep.append(w)
                if val is not None:
                    seen[name] = max(seen.get(name, -1), val)
            if len(keep) != len(si.on_wait):
                si.on_wait = keep


SUB = 64                 # v2 subchunk width
NSUB = NPAD // SUB       # 320 subchunk maxima per row
NSEL = 24                # subchunks selected per row (>= k+1 = 18 guaranteed)


def build_kernel_v2(r=R, npad=NPAD, chunk=CHUNK, block=BLOCK, kdim=KDIM,
                    sub=SUB, nsel=NSEL):
    """v2: hierarchical selection.

    Per 2048-col chunk: matmul -> PSUM f32; ScalarE casts to fp16 SBUF with
    per-row bias -x2_i (values become ~ -d2, so fp16 resolves the NN scale);
    VectorE reduce_max (fp16 2x mode) folds each 64-col subchunk to its max.
    Per block: 3 rounds of max8/max_index/match_replace over the [block, 320]
    subchunk-max array yield the top-24 subchunk ids + values per row.
    Host expands ids*64 -> candidate lists and rescores exactly.
    """
    nchunk = npad // chunk
    nblock = r // block
    nsub = npad // sub
    sub_per_chunk = chunk // sub
    rounds = nsel // 8
    assert nsel % 8 == 0

    nc = bass.Bass()
    f16, f32, u16 = mybir.dt.float16, mybir.dt.float32, mybir.dt.uint16
    qpT = nc.declare_dram_parameter("qpT", [kdim, r + npad], f16, isOutput=False)
    nx2q = nc.declare_dram_parameter("nx2q", [block, nblock], f32, isOutput=False)
    out_ids = nc.declare_dram_parameter(
        "ids", [block, nblock * nsel], u16, isOutput=True
    )
    out_vals = nc.declare_dram_parameter(
        "vals", [block, nblock * nsel], f16, isOutput=True
    )

    with TileContext(nc) as tc:
        with (
            tc.tile_pool(name="const", bufs=1) as cpool,
            tc.tile_pool(name="psum", bufs=2, space="PSUM") as ppool,
            tc.tile_pool(name="v16p", bufs=2) as vpool,
            tc.tile_pool(name="m1p", bufs=2) as mpool,
            tc.tile_pool(name="outs", bufs=1) as opool,
        ):
            qp_sb = cpool.tile([kdim, r + npad], f16)
            nc.sync.dma_start(out=qp_sb, in_=qpT[:, :])
            qT_sb = qp_sb[:, :r]
            pT_sb = qp_sb[:, r:]
            nx2_sb = cpool.tile([block, nblock], f32)
            nc.sync.dma_start(out=nx2_sb, in_=nx2q[:, :])

            ids_sb = opool.tile([block, nblock * nsel], u16)
            vals_sb = opool.tile([block, nblock * nsel], f16)

            for b in range(nblock):
                lhsT = qT_sb[:, b * block : (b + 1) * block]
                bias = nx2_sb[:, b : b + 1]
                v16 = vpool.tile([block, npad], f16)
                m1 = mpool.tile([block, nsub], f16)
                # Wait-carrier ops (overwritten by the real casts below; the
                # WAW edge orders them first on their engines). Each absorbs
                # one cross-proc wait — walrus allows only ~1 sync wait per
                # compute instruction — so the real casts keep <=1 wait: the
                # ACT one takes the v16-slot WAR release (and, on block 0,
                # the nx2q DMA wait); the DVE one takes the nx2q DMA wait for
                # the DVE-side cast of chunk nchunk-1.
                nc.scalar.activation(
                    v16[:, 0:1], bias, mybir.ActivationFunctionType.Copy
                )
                if b == 0:
                    nc.vector.tensor_copy(
                        v16[:, (nchunk - 1) * chunk : (nchunk - 1) * chunk + 1],
                        nx2_sb[:, 0:1],
                    )
                for ch in range(nchunk):
                    ps = ppool.tile([block, chunk], f32)
                    for s in range(chunk // 512):
                        col = ch * chunk + s * 512
                        nc.tensor.matmul(
                            ps[:, s * 512 : (s + 1) * 512],
                            lhsT=lhsT,
                            rhs=pT_sb[:, col : col + 512],
                            start=True,
                            stop=True,
                        )
                    dst = v16[:, ch * chunk : (ch + 1) * chunk]
                    if ch == nchunk - 1:
                        # balance: give one chunk's cast to the DVE
                        nc.vector.tensor_scalar(
                            dst, ps, bias, None, op0=mybir.AluOpType.add
                        )
                    else:
                        nc.scalar.activation(
                            dst, ps,
                            mybir.ActivationFunctionType.Identity,
                            bias=bias, scale=1.0,
                        )
                    nc.vector.reduce_max(
                        m1[:, ch * sub_per_chunk : (ch + 1) * sub_per_chunk],
                        dst.rearrange("p (c s) -> p c s", s=sub),
                        axis=mybir.AxisListType.X,
                    )
                for rd in range(rounds):
                    sl = slice((b * rounds + rd) * 8, (b * rounds + rd + 1) * 8)
                    nc.vector.max(out=vals_sb[:, sl], in_=m1)
                    nc.vector.max_index(
                        out=ids_sb[:, sl], in_max=vals_sb[:, sl], in_values=m1
                    )
                    if rd != rounds - 1:
                        nc.vector.match_replace(
                            out=m1, in_to_replace=vals_sb[:, sl],
                            in_values=m1, imm_value=-60000.0,
                        )
            nc.sync.dma_start(out=out_ids[:, :], in_=ids_sb)
            nc.sync.dma_start(out=out_vals[:, :], in_=vals_sb)

    _elide_redundant_waits(nc)
    _fix_tail_drain(nc)
    return nc


def _split_f16(a32):
    """fp32 array -> (hi, lo) fp16 pair with hi+lo ~ a to ~22 bits."""
    hi = a32.astype(np.float16)
    lo = (a32 - hi.astype(np.float32)).astype(np.float16)
    return hi, lo


def _prep_operands(pos32):
    """Build qT_all [14, N] and pT [14, NPAD] fp16 operand stacks.

    Row pairing (lhs row, rhs row) so that sum_k lhs[k,i]*rhs[k,j] =
    2*q_i.p_j - x2_j  with fp16 hi/lo products exact in f32:
      per coord c: (2qh, ph), (2qh, pl), (2ql, ph), (2ql, pl)   -> 12 rows
      (1, -x2h), (1, -x2l)                                      -> 2 rows
    """
    ph, pl = _split_f16(pos32)                      # [N,3] each
    x2_64 = np.sum(pos32.astype(np.float64) ** 2, axis=-1)
    x2h = x2_64.astype(np.float32).astype(np.float16)
    x2l = (x2_64 - x2h.astype(np.float64)).astype(np.float32).astype(np.float16)

    qT = np.zeros((KDIM, N), np.float16)
    pTf = np.zeros((KDIM, NPAD), np.float16)
    row = 0
    for c in range(3):
        qh2 = (2.0 * ph[:, c].astype(np.float32)).astype(np.float16)
        ql2 = (2.0 * pl[:, c].astype(np.float32)).astype(np.float16)
        for qrow, prow in ((qh2, ph[:, c]), (qh2, pl[:, c]),
                           (ql2, ph[:, c]), (ql2, pl[:, c])):
            qT[row] = qrow
            pTf[row, :N] = prow
            row += 1
    qT[row] = np.float16(1.0)
    pTf[row, :N] = -x2h
    pTf[row, N:] = np.float16(-60000.0)  # padded columns always lose
    row += 1
    qT[row] = np.float16(1.0)
    pTf[row, :N] = -x2l
    row += 1
    assert row == KDIM
    return qT, pTf


def _postprocess_v2(pos32, ids_all, vals_all, k):
    """ids/vals [N, NSEL] (subchunk ids + fp16 maxima of v' ~= -d2) ->
    (edge_index, dist), with exact f32 rescoring of the 24*SUB candidates."""
    n = pos32.shape[0]
    x2 = np.sum(pos32 * pos32, axis=-1)

    cand = (ids_all.astype(np.int32) * SUB)[:, :, None] + np.arange(
        SUB, dtype=np.int32
    )
    cand = cand.reshape(n, NSEL * SUB)

    selj = np.empty((n, k), np.int32)
    seld2 = np.empty((n, k), np.float32)
    cutoff = np.empty(n, np.float32)

    CH = 2000  # row-chunked to bound memory (~n/CH * NSEL*SUB*3 floats)
    rows_all = np.arange(n, dtype=np.int32)
    for r0 in range(0, n, CH):
        r1 = min(r0 + CH, n)
        cj = cand[r0:r1]
        valid = cj < n
        cjc = np.minimum(cj, n - 1)
        pj = pos32[cjc]
        dot = np.einsum("nd,ncd->nc", pos32[r0:r1], pj)
        d2r = x2[r0:r1, None] + x2[cjc] - 2.0 * dot
        d2r = np.where(valid, d2r, np.inf).astype(np.float32)
        d2m = np.where(cj == rows_all[r0:r1, None], np.inf, d2r)
        order = np.lexsort((cjc, d2m), axis=-1)
        sel = order[:, :k]
        selj[r0:r1] = np.take_along_axis(cj, sel, 1)
        seld2[r0:r1] = np.take_along_axis(d2m, sel, 1)
        part = np.partition(d2r, k, axis=1)
        cutoff[r0:r1] = part[:, k]

    # Safety: if the 24th subchunk max is at/above the (k+1)-th best
    # candidate (within fp16-quantization + matmul-error margin), unselected
    # subchunks could hide a true neighbor -> exact rescan of those rows.
    v24 = vals_all[:, NSEL - 1].astype(np.float32)
    margin = 0.02 + np.abs(v24) * 2.0**-9
    flagged = (-v24) <= cutoff + margin
    flagged |= ~np.isfinite(seld2[:, -1])

    for i in np.nonzero(flagged)[0]:
        d2i = x2[i] + x2 - 2.0 * (pos32 @ pos32[i])
        d2i[i] = np.inf
        oi = np.lexsort((np.arange(n), d2i))[:k]
        selj[i] = oi
        seld2[i] = d2i[oi]

    dist = np.sqrt(np.maximum(seld2, 0.0)).astype(np.float32)
    dst = np.repeat(np.arange(n, dtype=np.int32), k)
    edge_index = np.stack([selj.reshape(-1).astype(np.int32), dst])
    return edge_index, dist.reshape(-1)


def _postprocess(pos32, cand_all, k):
    """cand_all [N, NCHUNK*8] uint16 chunk-local indices -> (edge_index, dist)."""
    n = pos32.shape[0]
    x2 = np.sum(pos32 * pos32, axis=-1)  # f32, same op order as reference

    base = (np.arange(NCHUNK * 8, dtype=np.int32) // 8) * CHUNK
    gj = cand_all.astype(np.int32) + base[None, :]          # global ids
    valid = gj < n
    gjc = np.minimum(gj, n - 1)

    pj = pos32[gjc]                                          # [n, 80, 3]
    dot = np.einsum("nd,ncd->nc", pos32, pj)
    d2r = x2[:, None] + x2[gjc] - 2.0 * dot                  # rescored, f32
    d2r = np.where(valid, d2r, np.inf).astype(np.float32)

    rows = np.arange(n, dtype=np.int32)[:, None]
    d2m = np.where(gj == rows, np.inf, d2r)                  # self excluded

    order = np.lexsort((gjc, d2m), axis=-1)
    sel = order[:, :k]
    selj = np.take_along_axis(gj, sel, 1)
    seld2 = np.take_along_axis(d2m, sel, 1)

    # --- safety: rows where a saturated chunk could hide a missed neighbor
    part = np.partition(d2r, k, axis=1)                      # self included
    cutoff = part[:, k]                                      # (k+1)-th smallest
    chunk_max = d2r.reshape(n, NCHUNK, 8).max(axis=2)
    flagged = np.any(chunk_max <= cutoff[:, None] + MARGIN, axis=1)
    flagged |= ~np.isfinite(seld2[:, -1])

    for i in np.nonzero(flagged)[0]:
        d2i = x2[i] + x2 - 2.0 * (pos32 @ pos32[i])
        d2i[i] = np.inf
        oi = np.lexsort((np.arange(n), d2i))[:k]
        selj[i] = oi
        seld2[i] = d2i[oi]

    dist = np.sqrt(np.maximum(seld2, 0.0)).astype(np.float32)
    dst = np.repeat(np.arange(n, dtype=np.int32), k)
    edge_index = np.stack([selj.reshape(-1).astype(np.int32), dst])
    return edge_index, dist.reshape(-1)


def kernel(pos, k):
    global _last_exec_time_ns, _last_results
    k = int(k)
    pos32 = np.ascontiguousarray(np.asarray(pos), dtype=np.float32)
    assert pos32.shape == (N, 3), pos32.shape

    qT_all, pTf = _prep_operands(pos32)
    x2_64 = np.sum(pos32.astype(np.float64) ** 2, axis=-1)
    nx2 = (-x2_64).astype(np.float32)
    in_maps = []
    for c in range(NCORES):
        nx2q = np.ascontiguousarray(
            nx2[c * R : (c + 1) * R].reshape(R // BLOCK, BLOCK).T
        )
        in_maps.append(
            {
                "qpT": np.ascontiguousarray(
                    np.concatenate([qT_all[:, c * R : (c + 1) * R], pTf], axis=1)
                ),
                "nx2q": nx2q,
            }
        )

    nc = build_kernel_v2()
    res = run_bass_kernel_spmd(nc, in_maps, core_ids=list(range(NCORES)))
    _last_exec_time_ns = res.exec_time_ns
    _last_results = res

    def _decode(arr, w):
        # [block, nblock*w] -> [r, w] (row b*block+p = arr[p, b*w:(b+1)*w])
        a = arr.reshape(BLOCK, R // BLOCK, w)
        return a.transpose(1, 0, 2).reshape(R, w)

    ids_all = np.concatenate(
        [_decode(res.results[c]["ids"], NSEL) for c in range(NCORES)], axis=0
    )
    vals_all = np.concatenate(
        [_decode(res.results[c]["vals"], NSEL) for c in range(NCORES)], axis=0
    )
    return _postprocess_v2(pos32, ids_all, vals_all, k)


# revision 24
# speedup vs baseline: 1.6721x; 1.2677x over previous
"""Brute-force kNN graph (PyG knn_graph style) on 8 Trainium2 NeuronCores.

Strategy (data-parallel, row-sharded):
  - Each core owns 2500 query rows of the 20000x20000 distance matrix.
  - Device computes v_ij = 2*q_i.p_j - |p_j|^2 (row-shift of -d2, so per-row
    ranking equals nearest-neighbor ranking) via a K=14 split-fp16 matmul
    (fp16 hi/lo decomposition -> exact products accumulated in f32 PSUM,
    1 cycle/column on the PE vs 4 for fp32).
  - VectorE max (top-8) + max_index per 2048-column chunk -> 80 candidate
    indices per row (uint16, chunk-local).
  - Host rescores the 80 candidates exactly (same f32 Gram-trick ops as the
    reference), sorts by (d2, index) to match jax.lax.top_k tie-breaking,
    drops self, takes k. A saturated-chunk safety check (a chunk whose all 8
    returned candidates fall below the row's 18th-best + margin could hide
    more neighbors) routes rare rows to an exact full numpy rescan.
"""

import os
import sys

import numpy as np

for _p in ("/opt/trn_rl_repo", "/root/.axon_site/_ro/trn_rl_repo"):
    if os.path.isdir(_p) and _p not in sys.path:
        sys.path.append(_p)

from concourse import bass, mybir  # noqa: E402
from concourse.bass_utils import run_bass_kernel_spmd  # noqa: E402
from concourse.tile import TileContext  # noqa: E402
from concourse.tile_rust import add_dep_helper  # noqa: E402

N = 20000
KNN = 17
NCORES = 8
R = N // NCORES          # 2500 query rows per core
CHUNK = 2048             # candidate-chunk width (4 PSUM banks)
NCHUNK = 10
NPAD = CHUNK * NCHUNK    # 20480 columns (480 padded "far" points)
BLOCK = 125              # query rows per row-block (20 uniform blocks)
NBLOCK = R // BLOCK
KDIM = 14                # split-fp16 contraction depth
MARGIN = 0.1             # d2-units slack for the saturation check

_last_exec_time_ns = None
_last_results = None


def build_kernel(r=R, npad=NPAD, chunk=CHUNK, block=BLOCK, kdim=KDIM):
    """One-core Bass graph (SPMD across all cores).

    Inputs : qT [kdim, r] fp16 (stationary rows for this core's queries)
             pT [kdim, npad] fp16 (moving rows for all points)
    Output : cand [r, nchunk*8] uint16 (chunk-local top-8 indices)
    """
    nchunk = npad // chunk
    nblock = r // block
    assert r % block == 0 and npad % chunk == 0 and chunk % 512 == 0

    nc = bass.Bass()
    f16, f32, u16 = mybir.dt.float16, mybir.dt.float32, mybir.dt.uint16
    # Single combined input => one DMA transfer => one DMA semaphore for
    # consumers (walrus rejects instructions waiting on multiple DMA sems).
    qpT = nc.declare_dram_parameter("qpT", [kdim, r + npad], f16, isOutput=False)
    # Output stays in SBUF-native layout [block, nblock*nchunk*8]: row
    # (b*block+p)'s candidates live at cand[p, b*nchunk*8 : (b+1)*nchunk*8].
    # One contiguous DMA at the end; the host undoes the interleave.
    cand = nc.declare_dram_parameter(
        "cand", [block, nblock * nchunk * 8], u16, isOutput=True
    )

    with TileContext(nc) as tc:
        with (
            tc.tile_pool(name="const", bufs=1) as cpool,
            tc.tile_pool(name="psum", bufs=2, space="PSUM") as ppool,
            tc.tile_pool(name="outs", bufs=3) as opool,
            tc.tile_pool(name="mx", bufs=4) as mpool,
        ):
            qp_sb = cpool.tile([kdim, r + npad], f16)
            nc.sync.dma_start(out=qp_sb, in_=qpT[:, :])
            qT_sb = qp_sb[:, :r]
            pT_sb = qp_sb[:, r:]

            out_sb = opool.tile([block, nblock * nchunk * 8], u16)
            for b in range(nblock):
                lhsT = qT_sb[:, b * block : (b + 1) * block]
                for ch in range(nchunk):
                    ps = ppool.tile([block, chunk], f32)
                    for s in range(chunk // 512):
                        col = ch * chunk + s * 512
                        nc.tensor.matmul(
                            ps[:, s * 512 : (s + 1) * 512],
                            lhsT=lhsT,
                            rhs=pT_sb[:, col : col + 512],
                            start=True,
                            stop=True,
                        )
                    vmax = mpool.tile([block, 8], f32)
                    nc.vector.max(out=vmax, in_=ps)
                    nc.vector.max_index(
                        out=out_sb[:, (b * nchunk + ch) * 8 : (b * nchunk + ch + 1) * 8],
                        in_max=vmax,
                        in_values=ps,
                    )
            nc.sync.dma_start(out=cand[:, :], in_=out_sb)

    _elide_redundant_waits(nc)
    _fix_tail_drain(nc)
    return nc


def _fix_tail_drain(nc):
    """Walrus allows ~1 sync-wait per instruction; Tile's kernel-tail global
    drain carries one wait per logical proc. Engine/sequencer-completion
    waits are subsumed by the all-engine barrier that follows (each engine's
    own drain implies its queue is empty and its sem updates applied), so
    drop them. Spread the remaining DMA-queue waits one-per-instruction
    across the global drain and the barrier's per-engine drains (whose own
    `sem >= 0` waits are vacuous). Must not touch anything after the
    barrier's semaphore reset, so only the first barrier's drains are used.
    """
    import re

    insts = []
    for blk in nc.m.functions[0].blocks:
        insts.extend(blk.instructions)

    gi = None
    for k, inst in enumerate(insts):
        si = inst.sync_info
        if type(inst).__name__ == "InstDrain" and si and si.on_wait and len(si.on_wait) > 1:
            gi = k
    assert gi is not None, "global tail drain not found"
    drain = insts[gi]

    dma_waits = []
    for w in drain.sync_info.on_wait:
        name = w.ant_name or ""
        if re.fullmatch(r"DMA(HW|SW)\d+_\d+", name):
            dma_waits.append(w)
        # engine/sequencer completion waits: dropped (barrier-subsumed)
    drain.sync_info.on_wait = dma_waits[:1]
    rest = dma_waits[1:]

    k = gi + 1
    while rest and k < len(insts):
        inst = insts[k]
        k += 1
        if type(inst).__name__ != "InstDrain":
            continue
        si = inst.sync_info
        if si is None or (si.on_wait and any(w.wait_value for w in si.on_wait)):
            continue  # only reuse drains whose waits are vacuous (>= 0)
        si.on_wait = [rest.pop(0)]
    assert not rest, f"{len(rest)} tail DMA waits left unplaced"


def _elide_redundant_waits(nc):
    """Drop sync waits that are redundant under per-engine program order.

    Walrus rejects compute instructions carrying >1 sync-wait command. Tile
    emits (a) waits on the instruction's own engine-completion semaphore
    (engines execute their queue in order, so these always hold by issue
    time), and (b) waits already dominated by an equal-or-higher wait on the
    same semaphore earlier in the same engine's queue (queue heads block on
    waits, so later instructions inherit them). Both classes are removable.
    """
    import re

    enforced = {}  # engine name -> {sem name: max waited value}
    for blk in nc.m.functions[0].blocks:
        for inst in blk.instructions:
            si = inst.sync_info
            if si is None or not si.on_wait:
                continue
            if type(inst).__name__ in ("InstDrain", "InstEventSemaphore"):
                continue
            eng = inst.engine.name if inst.engine is not None else None
            if eng is None:
                continue
            is_dma = type(inst).__name__ == "InstDMACopy"
            seen = enforced.setdefault(eng, {})
            keep = []
            for w in si.on_wait:
                name = w.ant_name or ""
                val = w.wait_value
                if re.fullmatch(rf"{re.escape(eng)}_\d+", name):
                    continue  # own-engine completion sem
                if is_dma and re.fullmatch(r"DMA(HW|SW)\d+_\d+", name):
                    continue  # same-queue DMA ordering is in-order by HW
                if val is not None and seen.get(name, -1) >= val:
                    continue  # dominated by an earlier wait on this queue
                keep.append(w)
                if val is not None:
                    seen[name] = max(seen.get(name, -1), val)
            if len(keep) != len(si.on_wait):
                si.on_wait = keep


SUB = 64                 # v2 subchunk width
NSUB = NPAD // SUB       # 320 subchunk maxima per row
NSEL = 24                # subchunks selected per row (>= k+1 = 18 guaranteed)


def build_kernel_v2(r=R, npad=NPAD, chunk=CHUNK, block=BLOCK, kdim=KDIM,
                    sub=SUB, nsel=NSEL):
    """v2: hierarchical selection.

    Per 2048-col chunk: matmul -> PSUM f32; ScalarE casts to fp16 SBUF with
    per-row bias -x2_i (values become ~ -d2, so fp16 resolves the NN scale);
    VectorE reduce_max (fp16 2x mode) folds each 64-col subchunk to its max.
    Per block: 3 rounds of max8/max_index/match_replace over the [block, 320]
    subchunk-max array yield the top-24 subchunk ids + values per row.
    Host expands ids*64 -> candidate lists and rescores exactly.
    """
    nchunk = npad // chunk
    nblock = r // block
    nsub = npad // sub
    sub_per_chunk = chunk // sub
    rounds = nsel // 8
    assert nsel % 8 == 0

    nc = bass.Bass()
    f16, f32, u16 = mybir.dt.float16, mybir.dt.float32, mybir.dt.uint16
    qpT = nc.declare_dram_parameter("qpT", [kdim, r + npad], f16, isOutput=False)
    nx2q = nc.declare_dram_parameter("nx2q", [block, nblock], f32, isOutput=False)
    out_ids = nc.declare_dram_parameter(
        "ids", [block, nblock * nsel], u16, isOutput=True
    )
    out_vals = nc.declare_dram_parameter(
        "vals", [block, nblock * nsel], f16, isOutput=True
    )

    with TileContext(nc) as tc:
        with (
            tc.tile_pool(name="const", bufs=1) as cpool,
            tc.tile_pool(name="psum", bufs=2, space="PSUM") as ppool,
            tc.tile_pool(name="v16p", bufs=2) as vpool,
            tc.tile_pool(name="m1p", bufs=3) as mpool,
            tc.tile_pool(name="tree", bufs=2) as tpool,
            tc.tile_pool(name="outs", bufs=1) as opool,
        ):
            qp_sb = cpool.tile([kdim, r + npad], f16)
            nc.sync.dma_start(out=qp_sb, in_=qpT[:, :])
            qT_sb = qp_sb[:, :r]
            pT_sb = qp_sb[:, r:]
            nx2_sb = cpool.tile([block, nblock], f32)
            nc.sync.dma_start(out=nx2_sb, in_=nx2q[:, :])

            ids_sb = opool.tile([block, nblock * nsel], u16)
            vals_sb = opool.tile([block, nblock * nsel], f16)

            for b in range(nblock):
                lhsT = qT_sb[:, b * block : (b + 1) * block]
                bias = nx2_sb[:, b : b + 1]
                v16 = vpool.tile([block, npad], f16)
                m1 = mpool.tile([block, nsub], f16)
                # Wait-carrier ops (overwritten by the real casts below; the
                # WAW edge orders them first on their engines). Each absorbs
                # one cross-proc wait — walrus allows only ~1 sync wait per
                # compute instruction — so the real casts keep <=1 wait: the
                # ACT one takes the v16-slot WAR release (and, on block 0,
                # the nx2q DMA wait); the DVE one takes the nx2q DMA wait for
                # the DVE-side cast of chunk nchunk-1.
                nc.scalar.activation(
                    v16[:, 0:1], bias, mybir.ActivationFunctionType.Copy
                )
                if b == 0:
                    nc.vector.tensor_copy(
                        v16[:, (nchunk - 1) * chunk : (nchunk - 1) * chunk + 1],
                        nx2_sb[:, 0:1],
                    )
                for ch in range(nchunk):
                    ps = ppool.tile([block, chunk], f32)
                    for s in range(chunk // 512):
                        col = ch * chunk + s * 512
                        nc.tensor.matmul(
                            ps[:, s * 512 : (s + 1) * 512],
                            lhsT=lhsT,
                            rhs=pT_sb[:, col : col + 512],
                            start=True,
                            stop=True,
                        )
                    dst = v16[:, ch * chunk : (ch + 1) * chunk]
                    if ch == nchunk - 1:
                        # balance: give one chunk's cast to the DVE
                        nc.vector.tensor_scalar(
                            dst, ps, bias, None, op0=mybir.AluOpType.add
                        )
                    else:
                        nc.scalar.activation(
                            dst, ps,
                            mybir.ActivationFunctionType.Identity,
                            bias=bias, scale=1.0,
                        )
                    # Segmented reduce_max has no 2x-mode uop (measured 1x on
                    # HW); a tensor_tensor max tree keeps fp16 2x throughput.
                    cur = dst.rearrange("p (c s) -> p c s", s=sub)
                    w = sub
                    while w > 2:
                        nxt = tpool.tile(
                            [block, sub_per_chunk, w // 2], f16,
                            tag=f"tree{w}",
                        )
                        nc.vector.tensor_tensor(
                            out=nxt, in0=cur[:, :, : w // 2],
                            in1=cur[:, :, w // 2 :], op=mybir.AluOpType.max,
                        )
                        cur, w = nxt, w // 2
                    nc.vector.tensor_tensor(
                        out=m1[:, ch * sub_per_chunk : (ch + 1) * sub_per_chunk],
                        in0=cur[:, :, 0], in1=cur[:, :, 1],
                        op=mybir.AluOpType.max,
                    )
                for rd in range(rounds):
                    sl = slice((b * rounds + rd) * 8, (b * rounds + rd + 1) * 8)
                    nc.vector.max(out=vals_sb[:, sl], in_=m1)
                    nc.vector.max_index(
                        out=ids_sb[:, sl], in_max=vals_sb[:, sl], in_values=m1
                    )
                    if rd != rounds - 1:
                        nc.vector.match_replace(
                            out=m1, in_to_replace=vals_sb[:, sl],
                            in_values=m1, imm_value=-60000.0,
                        )
            nc.sync.dma_start(out=out_ids[:, :], in_=ids_sb)
            nc.sync.dma_start(out=out_vals[:, :], in_=vals_sb)

    _elide_redundant_waits(nc)
    _fix_tail_drain(nc)
    return nc


def _split_f16(a32):
    """fp32 array -> (hi, lo) fp16 pair with hi+lo ~ a to ~22 bits."""
    hi = a32.astype(np.float16)
    lo = (a32 - hi.astype(np.float32)).astype(np.float16)
    return hi, lo


def _prep_operands(pos32):
    """Build qT_all [14, N] and pT [14, NPAD] fp16 operand stacks.

    Row pairing (lhs row, rhs row) so that sum_k lhs[k,i]*rhs[k,j] =
    2*q_i.p_j - x2_j  with fp16 hi/lo products exact in f32:
      per coord c: (2qh, ph), (2qh, pl), (2ql, ph), (2ql, pl)   -> 12 rows
      (1, -x2h), (1, -x2l)                                      -> 2 rows
    """
    ph, pl = _split_f16(pos32)                      # [N,3] each
    x2_64 = np.sum(pos32.astype(np.float64) ** 2, axis=-1)
    x2h = x2_64.astype(np.float32).astype(np.float16)
    x2l = (x2_64 - x2h.astype(np.float64)).astype(np.float32).astype(np.float16)

    qT = np.zeros((KDIM, N), np.float16)
    pTf = np.zeros((KDIM, NPAD), np.float16)
    row = 0
    for c in range(3):
        qh2 = (2.0 * ph[:, c].astype(np.float32)).astype(np.float16)
        ql2 = (2.0 * pl[:, c].astype(np.float32)).astype(np.float16)
        for qrow, prow in ((qh2, ph[:, c]), (qh2, pl[:, c]),
                           (ql2, ph[:, c]), (ql2, pl[:, c])):
            qT[row] = qrow
            pTf[row, :N] = prow
            row += 1
    qT[row] = np.float16(1.0)
    pTf[row, :N] = -x2h
    pTf[row, N:] = np.float16(-60000.0)  # padded columns always lose
    row += 1
    qT[row] = np.float16(1.0)
    pTf[row, :N] = -x2l
    row += 1
    assert row == KDIM
    return qT, pTf


def _postprocess_v2(pos32, ids_all, vals_all, k):
    """ids/vals [N, NSEL] (subchunk ids + fp16 maxima of v' ~= -d2) ->
    (edge_index, dist), with exact f32 rescoring of the 24*SUB candidates."""
    n = pos32.shape[0]
    x2 = np.sum(pos32 * pos32, axis=-1)

    cand = (ids_all.astype(np.int32) * SUB)[:, :, None] + np.arange(
        SUB, dtype=np.int32
    )
    cand = cand.reshape(n, NSEL * SUB)

    selj = np.empty((n, k), np.int32)
    seld2 = np.empty((n, k), np.float32)
    cutoff = np.empty(n, np.float32)

    CH = 2000  # row-chunked to bound memory (~n/CH * NSEL*SUB*3 floats)
    rows_all = np.arange(n, dtype=np.int32)
    for r0 in range(0, n, CH):
        r1 = min(r0 + CH, n)
        cj = cand[r0:r1]
        valid = cj < n
        cjc = np.minimum(cj, n - 1)
        pj = pos32[cjc]
        dot = np.einsum("nd,ncd->nc", pos32[r0:r1], pj)
        d2r = x2[r0:r1, None] + x2[cjc] - 2.0 * dot
        d2r = np.where(valid, d2r, np.inf).astype(np.float32)
        d2m = np.where(cj == rows_all[r0:r1, None], np.inf, d2r)
        order = np.lexsort((cjc, d2m), axis=-1)
        sel = order[:, :k]
        selj[r0:r1] = np.take_along_axis(cj, sel, 1)
        seld2[r0:r1] = np.take_along_axis(d2m, sel, 1)
        part = np.partition(d2r, k, axis=1)
        cutoff[r0:r1] = part[:, k]

    # Safety: if the 24th subchunk max is at/above the (k+1)-th best
    # candidate (within fp16-quantization + matmul-error margin), unselected
    # subchunks could hide a true neighbor -> exact rescan of those rows.
    v24 = vals_all[:, NSEL - 1].astype(np.float32)
    margin = 0.02 + np.abs(v24) * 2.0**-9
    flagged = (-v24) <= cutoff + margin
    flagged |= ~np.isfinite(seld2[:, -1])

    for i in np.nonzero(flagged)[0]:
        d2i = x2[i] + x2 - 2.0 * (pos32 @ pos32[i])
        d2i[i] = np.inf
        oi = np.lexsort((np.arange(n), d2i))[:k]
        selj[i] = oi
        seld2[i] = d2i[oi]

    dist = np.sqrt(np.maximum(seld2, 0.0)).astype(np.float32)
    dst = np.repeat(np.arange(n, dtype=np.int32), k)
    edge_index = np.stack([selj.reshape(-1).astype(np.int32), dst])
    return edge_index, dist.reshape(-1)


def _postprocess(pos32, cand_all, k):
    """cand_all [N, NCHUNK*8] uint16 chunk-local indices -> (edge_index, dist)."""
    n = pos32.shape[0]
    x2 = np.sum(pos32 * pos32, axis=-1)  # f32, same op order as reference

    base = (np.arange(NCHUNK * 8, dtype=np.int32) // 8) * CHUNK
    gj = cand_all.astype(np.int32) + base[None, :]          # global ids
    valid = gj < n
    gjc = np.minimum(gj, n - 1)

    pj = pos32[gjc]                                          # [n, 80, 3]
    dot = np.einsum("nd,ncd->nc", pos32, pj)
    d2r = x2[:, None] + x2[gjc] - 2.0 * dot                  # rescored, f32
    d2r = np.where(valid, d2r, np.inf).astype(np.float32)

    rows = np.arange(n, dtype=np.int32)[:, None]
    d2m = np.where(gj == rows, np.inf, d2r)                  # self excluded

    order = np.lexsort((gjc, d2m), axis=-1)
    sel = order[:, :k]
    selj = np.take_along_axis(gj, sel, 1)
    seld2 = np.take_along_axis(d2m, sel, 1)

    # --- safety: rows where a saturated chunk could hide a missed neighbor
    part = np.partition(d2r, k, axis=1)                      # self included
    cutoff = part[:, k]                                      # (k+1)-th smallest
    chunk_max = d2r.reshape(n, NCHUNK, 8).max(axis=2)
    flagged = np.any(chunk_max <= cutoff[:, None] + MARGIN, axis=1)
    flagged |= ~np.isfinite(seld2[:, -1])

    for i in np.nonzero(flagged)[0]:
        d2i = x2[i] + x2 - 2.0 * (pos32 @ pos32[i])
        d2i[i] = np.inf
        oi = np.lexsort((np.arange(n), d2i))[:k]
        selj[i] = oi
        seld2[i] = d2i[oi]

    dist = np.sqrt(np.maximum(seld2, 0.0)).astype(np.float32)
    dst = np.repeat(np.arange(n, dtype=np.int32), k)
    edge_index = np.stack([selj.reshape(-1).astype(np.int32), dst])
    return edge_index, dist.reshape(-1)


def kernel(pos, k):
    global _last_exec_time_ns, _last_results
    k = int(k)
    pos32 = np.ascontiguousarray(np.asarray(pos), dtype=np.float32)
    assert pos32.shape == (N, 3), pos32.shape

    qT_all, pTf = _prep_operands(pos32)
    x2_64 = np.sum(pos32.astype(np.float64) ** 2, axis=-1)
    nx2 = (-x2_64).astype(np.float32)
    in_maps = []
    for c in range(NCORES):
        nx2q = np.ascontiguousarray(
            nx2[c * R : (c + 1) * R].reshape(R // BLOCK, BLOCK).T
        )
        in_maps.append(
            {
                "qpT": np.ascontiguousarray(
                    np.concatenate([qT_all[:, c * R : (c + 1) * R], pTf], axis=1)
                ),
                "nx2q": nx2q,
            }
        )

    nc = build_kernel_v2()
    res = run_bass_kernel_spmd(nc, in_maps, core_ids=list(range(NCORES)))
    _last_exec_time_ns = res.exec_time_ns
    _last_results = res

    def _decode(arr, w):
        # [block, nblock*w] -> [r, w] (row b*block+p = arr[p, b*w:(b+1)*w])
        a = arr.reshape(BLOCK, R // BLOCK, w)
        return a.transpose(1, 0, 2).reshape(R, w)

    ids_all = np.concatenate(
        [_decode(res.results[c]["ids"], NSEL) for c in range(NCORES)], axis=0
    )
    vals_all = np.concatenate(
        [_decode(res.results[c]["vals"], NSEL) for c in range(NCORES)], axis=0
    )
    return _postprocess_v2(pos32, ids_all, vals_all, k)


# revision 28
# speedup vs baseline: 1.8207x; 1.0889x over previous
"""Brute-force kNN graph (PyG knn_graph style) on 8 Trainium2 NeuronCores.

Strategy (data-parallel, row-sharded):
  - Each core owns 2500 query rows of the 20000x20000 distance matrix.
  - Device computes v_ij = 2*q_i.p_j - |p_j|^2 (row-shift of -d2, so per-row
    ranking equals nearest-neighbor ranking) via a K=14 split-fp16 matmul
    (fp16 hi/lo decomposition -> exact products accumulated in f32 PSUM,
    1 cycle/column on the PE vs 4 for fp32).
  - VectorE max (top-8) + max_index per 2048-column chunk -> 80 candidate
    indices per row (uint16, chunk-local).
  - Host rescores the 80 candidates exactly (same f32 Gram-trick ops as the
    reference), sorts by (d2, index) to match jax.lax.top_k tie-breaking,
    drops self, takes k. A saturated-chunk safety check (a chunk whose all 8
    returned candidates fall below the row's 18th-best + margin could hide
    more neighbors) routes rare rows to an exact full numpy rescan.
"""

import os
import sys

import numpy as np

for _p in ("/opt/trn_rl_repo", "/root/.axon_site/_ro/trn_rl_repo"):
    if os.path.isdir(_p) and _p not in sys.path:
        sys.path.append(_p)

from concourse import bass, mybir  # noqa: E402
from concourse.bass_utils import run_bass_kernel_spmd  # noqa: E402
from concourse.tile import TileContext  # noqa: E402
from concourse.tile_rust import add_dep_helper  # noqa: E402

N = 20000
KNN = 17
NCORES = 8
R = N // NCORES          # 2500 query rows per core
CHUNK = 2048             # candidate-chunk width (4 PSUM banks)
NCHUNK = 10
NPAD = CHUNK * NCHUNK    # 20480 columns (480 padded "far" points)
BLOCK = 125              # query rows per row-block (20 uniform blocks)
NBLOCK = R // BLOCK
KDIM = 14                # split-fp16 contraction depth
MARGIN = 0.1             # d2-units slack for the saturation check

_last_exec_time_ns = None
_last_results = None


def build_kernel(r=R, npad=NPAD, chunk=CHUNK, block=BLOCK, kdim=KDIM):
    """One-core Bass graph (SPMD across all cores).

    Inputs : qT [kdim, r] fp16 (stationary rows for this core's queries)
             pT [kdim, npad] fp16 (moving rows for all points)
    Output : cand [r, nchunk*8] uint16 (chunk-local top-8 indices)
    """
    nchunk = npad // chunk
    nblock = r // block
    assert r % block == 0 and npad % chunk == 0 and chunk % 512 == 0

    nc = bass.Bass()
    f16, f32, u16 = mybir.dt.float16, mybir.dt.float32, mybir.dt.uint16
    # Single combined input => one DMA transfer => one DMA semaphore for
    # consumers (walrus rejects instructions waiting on multiple DMA sems).
    qpT = nc.declare_dram_parameter("qpT", [kdim, r + npad], f16, isOutput=False)
    # Output stays in SBUF-native layout [block, nblock*nchunk*8]: row
    # (b*block+p)'s candidates live at cand[p, b*nchunk*8 : (b+1)*nchunk*8].
    # One contiguous DMA at the end; the host undoes the interleave.
    cand = nc.declare_dram_parameter(
        "cand", [block, nblock * nchunk * 8], u16, isOutput=True
    )

    with TileContext(nc) as tc:
        with (
            tc.tile_pool(name="const", bufs=1) as cpool,
            tc.tile_pool(name="psum", bufs=2, space="PSUM") as ppool,
            tc.tile_pool(name="outs", bufs=3) as opool,
            tc.tile_pool(name="mx", bufs=4) as mpool,
        ):
            qp_sb = cpool.tile([kdim, r + npad], f16)
            nc.sync.dma_start(out=qp_sb, in_=qpT[:, :])
            qT_sb = qp_sb[:, :r]
            pT_sb = qp_sb[:, r:]

            out_sb = opool.tile([block, nblock * nchunk * 8], u16)
            for b in range(nblock):
                lhsT = qT_sb[:, b * block : (b + 1) * block]
                for ch in range(nchunk):
                    ps = ppool.tile([block, chunk], f32)
                    for s in range(chunk // 512):
                        col = ch * chunk + s * 512
                        nc.tensor.matmul(
                            ps[:, s * 512 : (s + 1) * 512],
                            lhsT=lhsT,
                            rhs=pT_sb[:, col : col + 512],
                            start=True,
                            stop=True,
                        )
                    vmax = mpool.tile([block, 8], f32)
                    nc.vector.max(out=vmax, in_=ps)
                    nc.vector.max_index(
                        out=out_sb[:, (b * nchunk + ch) * 8 : (b * nchunk + ch + 1) * 8],
                        in_max=vmax,
                        in_values=ps,
                    )
            nc.sync.dma_start(out=cand[:, :], in_=out_sb)

    _elide_redundant_waits(nc)
    _dedup_ldweights(nc)
    _fix_tail_drain(nc)
    return nc


def _dedup_ldweights(nc):
    """Drop InstLdweights that reload the stationary operand already resident
    in the PE array. Tile lowers every matmul into an Ldweights+Matmult pair;
    within a row-block all 40 matmuls share one lhsT, so 39 reloads per block
    are pure overhead on the PE queue. Only wait/update-free duplicates whose
    weights AP and tile position match the previously retained load are
    removed (nothing else touches the PE between them)."""
    for blk in nc.m.functions[0].blocks:
        insts = blk.instructions
        keep = []
        last_key = None
        changed = False
        for inst in insts:
            if type(inst).__name__ == "InstLdweights":
                key = (str(inst.ins[0]), str(inst.tile_position))
                si = inst.sync_info
                clean = not (si and (si.on_wait or si.on_update))
                if clean and key == last_key:
                    changed = True
                    continue
                last_key = key
            keep.append(inst)
        if changed:
            blk.instructions = keep


def _fix_tail_drain(nc):
    """Walrus allows ~1 sync-wait per instruction; Tile's kernel-tail global
    drain carries one wait per logical proc. Engine/sequencer-completion
    waits are subsumed by the all-engine barrier that follows (each engine's
    own drain implies its queue is empty and its sem updates applied), so
    drop them. Spread the remaining DMA-queue waits one-per-instruction
    across the global drain and the barrier's per-engine drains (whose own
    `sem >= 0` waits are vacuous). Must not touch anything after the
    barrier's semaphore reset, so only the first barrier's drains are used.
    """
    import re

    insts = []
    for blk in nc.m.functions[0].blocks:
        insts.extend(blk.instructions)

    gi = None
    for k, inst in enumerate(insts):
        si = inst.sync_info
        if type(inst).__name__ == "InstDrain" and si and si.on_wait and len(si.on_wait) > 1:
            gi = k
    assert gi is not None, "global tail drain not found"
    drain = insts[gi]

    dma_waits = []
    for w in drain.sync_info.on_wait:
        name = w.ant_name or ""
        if re.fullmatch(r"DMA(HW|SW)\d+_\d+", name):
            dma_waits.append(w)
        # engine/sequencer completion waits: dropped (barrier-subsumed)
    drain.sync_info.on_wait = dma_waits[:1]
    rest = dma_waits[1:]

    k = gi + 1
    while rest and k < len(insts):
        inst = insts[k]
        k += 1
        if type(inst).__name__ != "InstDrain":
            continue
        si = inst.sync_info
        if si is None or (si.on_wait and any(w.wait_value for w in si.on_wait)):
            continue  # only reuse drains whose waits are vacuous (>= 0)
        si.on_wait = [rest.pop(0)]
    assert not rest, f"{len(rest)} tail DMA waits left unplaced"


def _elide_redundant_waits(nc):
    """Drop sync waits that are redundant under per-engine program order.

    Walrus rejects compute instructions carrying >1 sync-wait command. Tile
    emits (a) waits on the instruction's own engine-completion semaphore
    (engines execute their queue in order, so these always hold by issue
    time), and (b) waits already dominated by an equal-or-higher wait on the
    same semaphore earlier in the same engine's queue (queue heads block on
    waits, so later instructions inherit them). Both classes are removable.
    """
    import re

    enforced = {}  # engine name -> {sem name: max waited value}
    for blk in nc.m.functions[0].blocks:
        for inst in blk.instructions:
            si = inst.sync_info
            if si is None or not si.on_wait:
                continue
            if type(inst).__name__ in ("InstDrain", "InstEventSemaphore"):
                continue
            eng = inst.engine.name if inst.engine is not None else None
            if eng is None:
                continue
            is_dma = type(inst).__name__ == "InstDMACopy"
            seen = enforced.setdefault(eng, {})
            keep = []
            for w in si.on_wait:
                name = w.ant_name or ""
                val = w.wait_value
                if re.fullmatch(rf"{re.escape(eng)}_\d+", name):
                    continue  # own-engine completion sem
                if is_dma and re.fullmatch(r"DMA(HW|SW)\d+_\d+", name):
                    continue  # same-queue DMA ordering is in-order by HW
                if val is not None and seen.get(name, -1) >= val:
                    continue  # dominated by an earlier wait on this queue
                keep.append(w)
                if val is not None:
                    seen[name] = max(seen.get(name, -1), val)
            if len(keep) != len(si.on_wait):
                si.on_wait = keep


SUB = 64                 # v2 subchunk width
NSUB = NPAD // SUB       # 320 subchunk maxima per row
NSEL = 24                # subchunks selected per row (>= k+1 = 18 guaranteed)


def build_kernel_v2(r=R, npad=NPAD, chunk=CHUNK, block=BLOCK, kdim=KDIM,
                    sub=SUB, nsel=NSEL):
    """v2: hierarchical selection.

    Per 2048-col chunk: matmul -> PSUM f32; ScalarE casts to fp16 SBUF with
    per-row bias -x2_i (values become ~ -d2, so fp16 resolves the NN scale);
    VectorE reduce_max (fp16 2x mode) folds each 64-col subchunk to its max.
    Per block: 3 rounds of max8/max_index/match_replace over the [block, 320]
    subchunk-max array yield the top-24 subchunk ids + values per row.
    Host expands ids*64 -> candidate lists and rescores exactly.
    """
    nchunk = npad // chunk
    nblock = r // block
    nsub = npad // sub
    sub_per_chunk = chunk // sub
    rounds = nsel // 8
    assert nsel % 8 == 0

    nc = bass.Bass()
    f16, f32, u16 = mybir.dt.float16, mybir.dt.float32, mybir.dt.uint16
    qpT = nc.declare_dram_parameter("qpT", [kdim, r + npad], f16, isOutput=False)
    nx2q = nc.declare_dram_parameter("nx2q", [block, nblock], f32, isOutput=False)
    out_ids = nc.declare_dram_parameter(
        "ids", [block, nblock * nsel], u16, isOutput=True
    )
    out_vals = nc.declare_dram_parameter(
        "vals", [block, nblock * nsel], f16, isOutput=True
    )

    with TileContext(nc) as tc:
        with (
            tc.tile_pool(name="const", bufs=1) as cpool,
            tc.tile_pool(name="psum", bufs=2, space="PSUM") as ppool,
            tc.tile_pool(name="v16p", bufs=3) as vpool,
            tc.tile_pool(name="m1p", bufs=3) as mpool,
            tc.tile_pool(name="tree", bufs=2) as tpool,
            tc.tile_pool(name="outs", bufs=1) as opool,
        ):
            qp_sb = cpool.tile([kdim, r + npad], f16)
            nc.sync.dma_start(out=qp_sb, in_=qpT[:, :])
            qT_sb = qp_sb[:, :r]
            pT_sb = qp_sb[:, r:]
            nx2_sb = cpool.tile([block, nblock], f32)
            nc.sync.dma_start(out=nx2_sb, in_=nx2q[:, :])

            ids_sb = opool.tile([block, nblock * nsel], u16)
            vals_sb = opool.tile([block, nblock * nsel], f16)

            for b in range(nblock):
                lhsT = qT_sb[:, b * block : (b + 1) * block]
                bias = nx2_sb[:, b : b + 1]
                v16 = vpool.tile([block, npad], f16)
                m1 = mpool.tile([block, nsub], f16)
                # Wait-carrier op (overwritten by the real casts below; the
                # WAW edge orders it first on the ACT queue). It absorbs one
                # cross-proc wait — walrus allows only ~1 sync wait per
                # compute instruction — so the real casts keep <=1 wait: it
                # takes the v16-slot WAR release (and, on block 0, the nx2q
                # DMA wait).
                nc.scalar.activation(
                    v16[:, 0:1], bias, mybir.ActivationFunctionType.Copy
                )
                for ch in range(nchunk):
                    ps = ppool.tile([block, chunk], f32)
                    for s in range(chunk // 512):
                        col = ch * chunk + s * 512
                        nc.tensor.matmul(
                            ps[:, s * 512 : (s + 1) * 512],
                            lhsT=lhsT,
                            rhs=pT_sb[:, col : col + 512],
                            start=True,
                            stop=True,
                        )
                    dst = v16[:, ch * chunk : (ch + 1) * chunk]
                    nc.scalar.activation(
                        dst, ps,
                        mybir.ActivationFunctionType.Identity,
                        bias=bias, scale=1.0,
                    )
                    # Segmented reduce_max has no 2x-mode uop (measured 1x on
                    # HW); a tensor_tensor max tree keeps fp16 2x throughput.
                    cur = dst.rearrange("p (c s) -> p c s", s=sub)
                    w = sub
                    while w > 2:
                        nxt = tpool.tile(
                            [block, sub_per_chunk, w // 2], f16,
                            tag=f"tree{w}",
                        )
                        nc.vector.tensor_tensor(
                            out=nxt, in0=cur[:, :, : w // 2],
                            in1=cur[:, :, w // 2 :], op=mybir.AluOpType.max,
                        )
                        cur, w = nxt, w // 2
                    nc.vector.tensor_tensor(
                        out=m1[:, ch * sub_per_chunk : (ch + 1) * sub_per_chunk],
                        in0=cur[:, :, 0], in1=cur[:, :, 1],
                        op=mybir.AluOpType.max,
                    )
                for rd in range(rounds):
                    sl = slice((b * rounds + rd) * 8, (b * rounds + rd + 1) * 8)
                    nc.vector.max(out=vals_sb[:, sl], in_=m1)
                    nc.vector.max_index(
                        out=ids_sb[:, sl], in_max=vals_sb[:, sl], in_values=m1
                    )
                    if rd != rounds - 1:
                        nc.vector.match_replace(
                            out=m1, in_to_replace=vals_sb[:, sl],
                            in_values=m1, imm_value=-60000.0,
                        )
            nc.sync.dma_start(out=out_ids[:, :], in_=ids_sb)
            nc.sync.dma_start(out=out_vals[:, :], in_=vals_sb)

    _elide_redundant_waits(nc)
    _fix_tail_drain(nc)
    return nc


def _split_f16(a32):
    """fp32 array -> (hi, lo) fp16 pair with hi+lo ~ a to ~22 bits."""
    hi = a32.astype(np.float16)
    lo = (a32 - hi.astype(np.float32)).astype(np.float16)
    return hi, lo


def _prep_operands(pos32):
    """Build qT_all [14, N] and pT [14, NPAD] fp16 operand stacks.

    Row pairing (lhs row, rhs row) so that sum_k lhs[k,i]*rhs[k,j] =
    2*q_i.p_j - x2_j  with fp16 hi/lo products exact in f32:
      per coord c: (2qh, ph), (2qh, pl), (2ql, ph), (2ql, pl)   -> 12 rows
      (1, -x2h), (1, -x2l)                                      -> 2 rows
    """
    ph, pl = _split_f16(pos32)                      # [N,3] each
    x2_64 = np.sum(pos32.astype(np.float64) ** 2, axis=-1)
    x2h = x2_64.astype(np.float32).astype(np.float16)
    x2l = (x2_64 - x2h.astype(np.float64)).astype(np.float32).astype(np.float16)

    qT = np.zeros((KDIM, N), np.float16)
    pTf = np.zeros((KDIM, NPAD), np.float16)
    row = 0
    for c in range(3):
        qh2 = (2.0 * ph[:, c].astype(np.float32)).astype(np.float16)
        ql2 = (2.0 * pl[:, c].astype(np.float32)).astype(np.float16)
        for qrow, prow in ((qh2, ph[:, c]), (qh2, pl[:, c]),
                           (ql2, ph[:, c]), (ql2, pl[:, c])):
            qT[row] = qrow
            pTf[row, :N] = prow
            row += 1
    qT[row] = np.float16(1.0)
    pTf[row, :N] = -x2h
    pTf[row, N:] = np.float16(-60000.0)  # padded columns always lose
    row += 1
    qT[row] = np.float16(1.0)
    pTf[row, :N] = -x2l
    row += 1
    assert row == KDIM
    return qT, pTf


def _postprocess_v2(pos32, ids_all, vals_all, k):
    """ids/vals [N, NSEL] (subchunk ids + fp16 maxima of v' ~= -d2) ->
    (edge_index, dist), with exact f32 rescoring of the 24*SUB candidates."""
    n = pos32.shape[0]
    x2 = np.sum(pos32 * pos32, axis=-1)

    cand = (ids_all.astype(np.int32) * SUB)[:, :, None] + np.arange(
        SUB, dtype=np.int32
    )
    cand = cand.reshape(n, NSEL * SUB)

    selj = np.empty((n, k), np.int32)
    seld2 = np.empty((n, k), np.float32)
    cutoff = np.empty(n, np.float32)

    CH = 2000  # row-chunked to bound memory (~n/CH * NSEL*SUB*3 floats)
    rows_all = np.arange(n, dtype=np.int32)
    for r0 in range(0, n, CH):
        r1 = min(r0 + CH, n)
        cj = cand[r0:r1]
        valid = cj < n
        cjc = np.minimum(cj, n - 1)
        pj = pos32[cjc]
        dot = np.einsum("nd,ncd->nc", pos32[r0:r1], pj)
        d2r = x2[r0:r1, None] + x2[cjc] - 2.0 * dot
        d2r = np.where(valid, d2r, np.inf).astype(np.float32)
        d2m = np.where(cj == rows_all[r0:r1, None], np.inf, d2r)
        order = np.lexsort((cjc, d2m), axis=-1)
        sel = order[:, :k]
        selj[r0:r1] = np.take_along_axis(cj, sel, 1)
        seld2[r0:r1] = np.take_along_axis(d2m, sel, 1)
        part = np.partition(d2r, k, axis=1)
        cutoff[r0:r1] = part[:, k]

    # Safety: if the 24th subchunk max is at/above the (k+1)-th best
    # candidate (within fp16-quantization + matmul-error margin), unselected
    # subchunks could hide a true neighbor -> exact rescan of those rows.
    v24 = vals_all[:, NSEL - 1].astype(np.float32)
    margin = 0.02 + np.abs(v24) * 2.0**-9
    flagged = (-v24) <= cutoff + margin
    flagged |= ~np.isfinite(seld2[:, -1])

    for i in np.nonzero(flagged)[0]:
        d2i = x2[i] + x2 - 2.0 * (pos32 @ pos32[i])
        d2i[i] = np.inf
        oi = np.lexsort((np.arange(n), d2i))[:k]
        selj[i] = oi
        seld2[i] = d2i[oi]

    dist = np.sqrt(np.maximum(seld2, 0.0)).astype(np.float32)
    dst = np.repeat(np.arange(n, dtype=np.int32), k)
    edge_index = np.stack([selj.reshape(-1).astype(np.int32), dst])
    return edge_index, dist.reshape(-1)


def _postprocess(pos32, cand_all, k):
    """cand_all [N, NCHUNK*8] uint16 chunk-local indices -> (edge_index, dist)."""
    n = pos32.shape[0]
    x2 = np.sum(pos32 * pos32, axis=-1)  # f32, same op order as reference

    base = (np.arange(NCHUNK * 8, dtype=np.int32) // 8) * CHUNK
    gj = cand_all.astype(np.int32) + base[None, :]          # global ids
    valid = gj < n
    gjc = np.minimum(gj, n - 1)

    pj = pos32[gjc]                                          # [n, 80, 3]
    dot = np.einsum("nd,ncd->nc", pos32, pj)
    d2r = x2[:, None] + x2[gjc] - 2.0 * dot                  # rescored, f32
    d2r = np.where(valid, d2r, np.inf).astype(np.float32)

    rows = np.arange(n, dtype=np.int32)[:, None]
    d2m = np.where(gj == rows, np.inf, d2r)                  # self excluded

    order = np.lexsort((gjc, d2m), axis=-1)
    sel = order[:, :k]
    selj = np.take_along_axis(gj, sel, 1)
    seld2 = np.take_along_axis(d2m, sel, 1)

    # --- safety: rows where a saturated chunk could hide a missed neighbor
    part = np.partition(d2r, k, axis=1)                      # self included
    cutoff = part[:, k]                                      # (k+1)-th smallest
    chunk_max = d2r.reshape(n, NCHUNK, 8).max(axis=2)
    flagged = np.any(chunk_max <= cutoff[:, None] + MARGIN, axis=1)
    flagged |= ~np.isfinite(seld2[:, -1])

    for i in np.nonzero(flagged)[0]:
        d2i = x2[i] + x2 - 2.0 * (pos32 @ pos32[i])
        d2i[i] = np.inf
        oi = np.lexsort((np.arange(n), d2i))[:k]
        selj[i] = oi
        seld2[i] = d2i[oi]

    dist = np.sqrt(np.maximum(seld2, 0.0)).astype(np.float32)
    dst = np.repeat(np.arange(n, dtype=np.int32), k)
    edge_index = np.stack([selj.reshape(-1).astype(np.int32), dst])
    return edge_index, dist.reshape(-1)


def kernel(pos, k):
    global _last_exec_time_ns, _last_results
    k = int(k)
    pos32 = np.ascontiguousarray(np.asarray(pos), dtype=np.float32)
    assert pos32.shape == (N, 3), pos32.shape

    qT_all, pTf = _prep_operands(pos32)
    x2_64 = np.sum(pos32.astype(np.float64) ** 2, axis=-1)
    nx2 = (-x2_64).astype(np.float32)
    in_maps = []
    for c in range(NCORES):
        nx2q = np.ascontiguousarray(
            nx2[c * R : (c + 1) * R].reshape(R // BLOCK, BLOCK).T
        )
        in_maps.append(
            {
                "qpT": np.ascontiguousarray(
                    np.concatenate([qT_all[:, c * R : (c + 1) * R], pTf], axis=1)
                ),
                "nx2q": nx2q,
            }
        )

    nc = build_kernel_v2()
    res = run_bass_kernel_spmd(nc, in_maps, core_ids=list(range(NCORES)))
    _last_exec_time_ns = res.exec_time_ns
    _last_results = res

    def _decode(arr, w):
        # [block, nblock*w] -> [r, w] (row b*block+p = arr[p, b*w:(b+1)*w])
        a = arr.reshape(BLOCK, R // BLOCK, w)
        return a.transpose(1, 0, 2).reshape(R, w)

    ids_all = np.concatenate(
        [_decode(res.results[c]["ids"], NSEL) for c in range(NCORES)], axis=0
    )
    vals_all = np.concatenate(
        [_decode(res.results[c]["vals"], NSEL) for c in range(NCORES)], axis=0
    )
    return _postprocess_v2(pos32, ids_all, vals_all, k)


# revision 34
# speedup vs baseline: 1.9985x; 1.0976x over previous
"""Brute-force kNN graph (PyG knn_graph style) on 8 Trainium2 NeuronCores.

Strategy (data-parallel, row-sharded):
  - Each core owns 2500 query rows of the 20000x20000 distance matrix.
  - Device computes v_ij = 2*q_i.p_j - |p_j|^2 (row-shift of -d2, so per-row
    ranking equals nearest-neighbor ranking) via a K=14 split-fp16 matmul
    (fp16 hi/lo decomposition -> exact products accumulated in f32 PSUM,
    1 cycle/column on the PE vs 4 for fp32).
  - VectorE max (top-8) + max_index per 2048-column chunk -> 80 candidate
    indices per row (uint16, chunk-local).
  - Host rescores the 80 candidates exactly (same f32 Gram-trick ops as the
    reference), sorts by (d2, index) to match jax.lax.top_k tie-breaking,
    drops self, takes k. A saturated-chunk safety check (a chunk whose all 8
    returned candidates fall below the row's 18th-best + margin could hide
    more neighbors) routes rare rows to an exact full numpy rescan.
"""

import os
import sys

import numpy as np

for _p in ("/opt/trn_rl_repo", "/root/.axon_site/_ro/trn_rl_repo"):
    if os.path.isdir(_p) and _p not in sys.path:
        sys.path.append(_p)

try:
    import antenv.axon_hooks as _ah  # noqa: F401
except ImportError:
    # This container's antenv lacks axon_hooks; concourse's trace path
    # imports it unconditionally. Provide a no-op hook so a stray
    # BASS_TRACE=1 cannot crash the run (tracing just degrades).
    import types as _types

    _m = _types.ModuleType("antenv.axon_hooks")
    _m._hook = None
    _m.get_axon_ntff_profile_hook = lambda: _m._hook
    _m.set_axon_ntff_profile_hook = lambda h: setattr(_m, "_hook", h)
    sys.modules["antenv.axon_hooks"] = _m

from concourse import bass, mybir  # noqa: E402
from concourse.bass_utils import run_bass_kernel_spmd  # noqa: E402
from concourse.tile import TileContext  # noqa: E402

N = 20000
KNN = 17
NCORES = 8
R = N // NCORES          # 2500 query rows per core
CHUNK = 2048             # candidate-chunk width (4 PSUM banks)
NCHUNK = 10
NPAD = CHUNK * NCHUNK    # 20480 columns (480 padded "far" points)
BLOCK = 125              # query rows per row-block (20 uniform blocks)
NBLOCK = R // BLOCK
KDIM = 14                # split-fp16 contraction depth
MARGIN = 0.1             # d2-units slack for the saturation check

_last_exec_time_ns = None
_last_results = None


def build_kernel(r=R, npad=NPAD, chunk=CHUNK, block=BLOCK, kdim=KDIM):
    """One-core Bass graph (SPMD across all cores).

    Inputs : qT [kdim, r] fp16 (stationary rows for this core's queries)
             pT [kdim, npad] fp16 (moving rows for all points)
    Output : cand [r, nchunk*8] uint16 (chunk-local top-8 indices)
    """
    nchunk = npad // chunk
    nblock = r // block
    assert r % block == 0 and npad % chunk == 0 and chunk % 512 == 0

    nc = bass.Bass()
    f16, f32, u16 = mybir.dt.float16, mybir.dt.float32, mybir.dt.uint16
    # Single combined input => one DMA transfer => one DMA semaphore for
    # consumers (walrus rejects instructions waiting on multiple DMA sems).
    qpT = nc.declare_dram_parameter("qpT", [kdim, r + npad], f16, isOutput=False)
    # Output stays in SBUF-native layout [block, nblock*nchunk*8]: row
    # (b*block+p)'s candidates live at cand[p, b*nchunk*8 : (b+1)*nchunk*8].
    # One contiguous DMA at the end; the host undoes the interleave.
    cand = nc.declare_dram_parameter(
        "cand", [block, nblock * nchunk * 8], u16, isOutput=True
    )

    with TileContext(nc) as tc:
        with (
            tc.tile_pool(name="const", bufs=1) as cpool,
            tc.tile_pool(name="psum", bufs=2, space="PSUM") as ppool,
            tc.tile_pool(name="outs", bufs=3) as opool,
            tc.tile_pool(name="mx", bufs=4) as mpool,
        ):
            qp_sb = cpool.tile([kdim, r + npad], f16)
            nc.sync.dma_start(out=qp_sb, in_=qpT[:, :])
            qT_sb = qp_sb[:, :r]
            pT_sb = qp_sb[:, r:]

            out_sb = opool.tile([block, nblock * nchunk * 8], u16)
            for b in range(nblock):
                lhsT = qT_sb[:, b * block : (b + 1) * block]
                for ch in range(nchunk):
                    ps = ppool.tile([block, chunk], f32)
                    for s in range(chunk // 512):
                        col = ch * chunk + s * 512
                        nc.tensor.matmul(
                            ps[:, s * 512 : (s + 1) * 512],
                            lhsT=lhsT,
                            rhs=pT_sb[:, col : col + 512],
                            start=True,
                            stop=True,
                        )
                    vmax = mpool.tile([block, 8], f32)
                    nc.vector.max(out=vmax, in_=ps)
                    nc.vector.max_index(
                        out=out_sb[:, (b * nchunk + ch) * 8 : (b * nchunk + ch + 1) * 8],
                        in_max=vmax,
                        in_values=ps,
                    )
            nc.sync.dma_start(out=cand[:, :], in_=out_sb)

    _elide_redundant_waits(nc)
    if os.environ.get("KNN_NO_LDW_DEDUP") != "1":
        _dedup_ldweights(nc)
    _fix_tail_drain(nc)
    return nc


def _dedup_ldweights(nc):
    """Drop InstLdweights that reload the stationary operand already resident
    in the PE array. Tile lowers every matmul into an Ldweights+Matmult pair;
    within a row-block all 40 matmuls share one lhsT, so 39 reloads per block
    are pure overhead on the PE queue. Only wait/update-free duplicates whose
    weights AP and tile position match the previously retained load are
    removed (nothing else touches the PE between them)."""
    for blk in nc.m.functions[0].blocks:
        insts = blk.instructions
        keep = []
        last_key = None
        changed = False
        for inst in insts:
            if type(inst).__name__ == "InstLdweights":
                key = (str(inst.ins[0]), str(inst.tile_position))
                si = inst.sync_info
                clean = not (si and (si.on_wait or si.on_update))
                if clean and key == last_key:
                    changed = True
                    continue
                last_key = key
            keep.append(inst)
        if changed:
            blk.instructions = keep


def _fix_tail_drain(nc):
    """Walrus allows ~1 sync-wait per instruction; Tile's kernel-tail global
    drain carries one wait per logical proc. Engine/sequencer-completion
    waits are subsumed by the all-engine barrier that follows (each engine's
    own drain implies its queue is empty and its sem updates applied), so
    drop them. Spread the remaining DMA-queue waits one-per-instruction
    across the global drain and the barrier's per-engine drains (whose own
    `sem >= 0` waits are vacuous). Must not touch anything after the
    barrier's semaphore reset, so only the first barrier's drains are used.
    """
    import re

    insts = []
    for blk in nc.m.functions[0].blocks:
        insts.extend(blk.instructions)

    gi = None
    for k, inst in enumerate(insts):
        si = inst.sync_info
        if type(inst).__name__ == "InstDrain" and si and si.on_wait and len(si.on_wait) > 1:
            gi = k
    assert gi is not None, "global tail drain not found"
    drain = insts[gi]

    dma_waits = []
    for w in drain.sync_info.on_wait:
        name = w.ant_name or ""
        if re.fullmatch(r"DMA(HW|SW)\d+_\d+", name):
            dma_waits.append(w)
        # engine/sequencer completion waits: dropped (barrier-subsumed)
    drain.sync_info.on_wait = dma_waits[:1]
    rest = dma_waits[1:]

    k = gi + 1
    while rest and k < len(insts):
        inst = insts[k]
        k += 1
        if type(inst).__name__ != "InstDrain":
            continue
        si = inst.sync_info
        if si is None or (si.on_wait and any(w.wait_value for w in si.on_wait)):
            continue  # only reuse drains whose waits are vacuous (>= 0)
        si.on_wait = [rest.pop(0)]
    assert not rest, f"{len(rest)} tail DMA waits left unplaced"


def _elide_redundant_waits(nc):
    """Drop sync waits that are redundant under per-engine program order.

    Walrus rejects compute instructions carrying >1 sync-wait command. Tile
    emits (a) waits on the instruction's own engine-completion semaphore
    (engines execute their queue in order, so these always hold by issue
    time), and (b) waits already dominated by an equal-or-higher wait on the
    same semaphore earlier in the same engine's queue (queue heads block on
    waits, so later instructions inherit them). Both classes are removable.
    """
    import re

    enforced = {}  # engine name -> {sem name: max waited value}
    for blk in nc.m.functions[0].blocks:
        for inst in blk.instructions:
            si = inst.sync_info
            if si is None or not si.on_wait:
                continue
            if type(inst).__name__ in ("InstDrain", "InstEventSemaphore"):
                continue
            eng = inst.engine.name if inst.engine is not None else None
            if eng is None:
                continue
            is_dma = type(inst).__name__ == "InstDMACopy"
            seen = enforced.setdefault(eng, {})
            keep = []
            for w in si.on_wait:
                name = w.ant_name or ""
                val = w.wait_value
                if re.fullmatch(rf"{re.escape(eng)}_\d+", name):
                    continue  # own-engine completion sem
                if is_dma and re.fullmatch(r"DMA(HW|SW)\d+_\d+", name):
                    continue  # same-queue DMA ordering is in-order by HW
                if val is not None and seen.get(name, -1) >= val:
                    continue  # dominated by an earlier wait on this queue
                keep.append(w)
                if val is not None:
                    seen[name] = max(seen.get(name, -1), val)
            if len(keep) != len(si.on_wait):
                si.on_wait = keep


SUB = 64                 # v2 subchunk width
NSUB = NPAD // SUB       # 320 subchunk maxima per row
NSEL = 24                # subchunks selected per row (>= k+1 = 18 guaranteed)


def build_kernel_v2(r=R, npad=NPAD, chunk=CHUNK, block=BLOCK, kdim=KDIM,
                    sub=SUB, nsel=NSEL):
    """v2: hierarchical selection.

    Per 2048-col chunk: matmul -> PSUM f32; ScalarE casts to fp16 SBUF with
    per-row bias -x2_i (values become ~ -d2, so fp16 resolves the NN scale);
    VectorE reduce_max (fp16 2x mode) folds each 64-col subchunk to its max.
    Per block: 3 rounds of max8/max_index/match_replace over the [block, 320]
    subchunk-max array yield the top-24 subchunk ids + values per row.
    Host expands ids*64 -> candidate lists and rescores exactly.
    """
    nchunk = npad // chunk
    nblock = r // block
    nsub = npad // sub
    sub_per_chunk = chunk // sub
    rounds = nsel // 8
    assert nsel % 8 == 0

    nc = bass.Bass()
    f16, f32, u16 = mybir.dt.float16, mybir.dt.float32, mybir.dt.uint16
    qpT = nc.declare_dram_parameter("qpT", [kdim, r + npad], f16, isOutput=False)
    nx2q = nc.declare_dram_parameter("nx2q", [block, nblock], f32, isOutput=False)
    out_ids = nc.declare_dram_parameter(
        "ids", [block, nblock * nsel], u16, isOutput=True
    )
    out_vals = nc.declare_dram_parameter(
        "vals", [block, nblock * nsel], f16, isOutput=True
    )

    with TileContext(nc) as tc:
        with (
            tc.tile_pool(name="const", bufs=1) as cpool,
            tc.tile_pool(name="psum", bufs=2, space="PSUM") as ppool,
            tc.tile_pool(name="v16p", bufs=2) as vpool,
            tc.tile_pool(name="m1p", bufs=3) as mpool,
            tc.tile_pool(name="tree", bufs=2) as tpool,
            tc.tile_pool(name="outs", bufs=1) as opool,
        ):
            qp_sb = cpool.tile([kdim, r + npad], f16)
            nc.sync.dma_start(out=qp_sb, in_=qpT[:, :])
            qT_sb = qp_sb[:, :r]
            pT_sb = qp_sb[:, r:]
            nx2_sb = cpool.tile([block, nblock], f32)
            nc.sync.dma_start(out=nx2_sb, in_=nx2q[:, :])

            ids_sb = opool.tile([block, nblock * nsel], u16)
            vals_sb = opool.tile([block, nblock * nsel], f16)

            for b in range(nblock):
                lhsT = qT_sb[:, b * block : (b + 1) * block]
                bias = nx2_sb[:, b : b + 1]
                v16 = vpool.tile([block, npad], f16)
                m1 = mpool.tile([block, nsub], f16)
                # Wait-carrier ops (overwritten by the real casts below; the
                # WAW edge orders them first on their engines). Each absorbs
                # one cross-proc wait — walrus allows only ~1 sync wait per
                # compute instruction — so the real casts keep <=1 wait.
                nc.scalar.activation(
                    v16[:, 0:1], bias, mybir.ActivationFunctionType.Copy
                )
                if b == 0:
                    nc.vector.tensor_copy(
                        v16[:, (nchunk - 1) * chunk : (nchunk - 1) * chunk + 1],
                        nx2_sb[:, 0:1],
                    )
                for ch in range(nchunk):
                    ps = ppool.tile([block, chunk], f32)
                    for s in range(chunk // 512):
                        col = ch * chunk + s * 512
                        nc.tensor.matmul(
                            ps[:, s * 512 : (s + 1) * 512],
                            lhsT=lhsT,
                            rhs=pT_sb[:, col : col + 512],
                            start=True,
                            stop=True,
                        )
                    dst = v16[:, ch * chunk : (ch + 1) * chunk]
                    if ch == nchunk - 1:
                        # balance: give one chunk's cast to the DVE
                        nc.vector.tensor_scalar(
                            dst, ps, bias, None, op0=mybir.AluOpType.add
                        )
                    else:
                        nc.scalar.activation(
                            dst, ps,
                            mybir.ActivationFunctionType.Identity,
                            bias=bias, scale=1.0,
                        )
                    # Segmented reduce_max has no 2x-mode uop (measured 1x on
                    # HW); a tensor_tensor max tree keeps fp16 2x throughput.
                    cur = dst.rearrange("p (c s) -> p c s", s=sub)
                    w = sub
                    while w > 2:
                        nxt = tpool.tile(
                            [block, sub_per_chunk, w // 2], f16,
                            tag=f"tree{w}",
                        )
                        nc.vector.tensor_tensor(
                            out=nxt, in0=cur[:, :, : w // 2],
                            in1=cur[:, :, w // 2 :], op=mybir.AluOpType.max,
                        )
                        cur, w = nxt, w // 2
                    nc.vector.tensor_tensor(
                        out=m1[:, ch * sub_per_chunk : (ch + 1) * sub_per_chunk],
                        in0=cur[:, :, 0], in1=cur[:, :, 1],
                        op=mybir.AluOpType.max,
                    )
                for rd in range(rounds):
                    sl = slice((b * rounds + rd) * 8, (b * rounds + rd + 1) * 8)
                    nc.vector.max(out=vals_sb[:, sl], in_=m1)
                    nc.vector.max_index(
                        out=ids_sb[:, sl], in_max=vals_sb[:, sl], in_values=m1
                    )
                    if rd != rounds - 1:
                        nc.vector.match_replace(
                            out=m1, in_to_replace=vals_sb[:, sl],
                            in_values=m1, imm_value=-60000.0,
                        )
            nc.sync.dma_start(out=out_ids[:, :], in_=ids_sb)
            nc.sync.dma_start(out=out_vals[:, :], in_=vals_sb)

    _elide_redundant_waits(nc)
    _fix_tail_drain(nc)
    return nc


def _split_f16(a32):
    """fp32 array -> (hi, lo) fp16 pair with hi+lo ~ a to ~22 bits."""
    hi = a32.astype(np.float16)
    lo = (a32 - hi.astype(np.float32)).astype(np.float16)
    return hi, lo


def _prep_operands(pos32):
    """Build qT_all [14, N] and pT [14, NPAD] fp16 operand stacks.

    Row pairing (lhs row, rhs row) so that sum_k lhs[k,i]*rhs[k,j] =
    2*q_i.p_j - x2_j  with fp16 hi/lo products exact in f32:
      per coord c: (2qh, ph), (2qh, pl), (2ql, ph), (2ql, pl)   -> 12 rows
      (1, -x2h), (1, -x2l)                                      -> 2 rows
    """
    ph, pl = _split_f16(pos32)                      # [N,3] each
    x2_64 = np.sum(pos32.astype(np.float64) ** 2, axis=-1)
    x2h = x2_64.astype(np.float32).astype(np.float16)
    x2l = (x2_64 - x2h.astype(np.float64)).astype(np.float32).astype(np.float16)

    qT = np.zeros((KDIM, N), np.float16)
    pTf = np.zeros((KDIM, NPAD), np.float16)
    row = 0
    for c in range(3):
        qh2 = (2.0 * ph[:, c].astype(np.float32)).astype(np.float16)
        ql2 = (2.0 * pl[:, c].astype(np.float32)).astype(np.float16)
        for qrow, prow in ((qh2, ph[:, c]), (qh2, pl[:, c]),
                           (ql2, ph[:, c]), (ql2, pl[:, c])):
            qT[row] = qrow
            pTf[row, :N] = prow
            row += 1
    qT[row] = np.float16(1.0)
    pTf[row, :N] = -x2h
    pTf[row, N:] = np.float16(-60000.0)  # padded columns always lose
    row += 1
    qT[row] = np.float16(1.0)
    pTf[row, :N] = -x2l
    row += 1
    assert row == KDIM
    return qT, pTf


def _postprocess_v2(pos32, ids_all, vals_all, k):
    """ids/vals [N, NSEL] (subchunk ids + fp16 maxima of v' ~= -d2) ->
    (edge_index, dist), with exact f32 rescoring of the 24*SUB candidates."""
    n = pos32.shape[0]
    x2 = np.sum(pos32 * pos32, axis=-1)

    cand = (ids_all.astype(np.int32) * SUB)[:, :, None] + np.arange(
        SUB, dtype=np.int32
    )
    cand = cand.reshape(n, NSEL * SUB)

    selj = np.empty((n, k), np.int32)
    seld2 = np.empty((n, k), np.float32)
    cutoff = np.empty(n, np.float32)

    CH = 2000  # row-chunked to bound memory (~n/CH * NSEL*SUB*3 floats)
    rows_all = np.arange(n, dtype=np.int32)
    for r0 in range(0, n, CH):
        r1 = min(r0 + CH, n)
        cj = cand[r0:r1]
        valid = cj < n
        cjc = np.minimum(cj, n - 1)
        pj = pos32[cjc]
        dot = np.einsum("nd,ncd->nc", pos32[r0:r1], pj)
        d2r = x2[r0:r1, None] + x2[cjc] - 2.0 * dot
        d2r = np.where(valid, d2r, np.inf).astype(np.float32)
        d2m = np.where(cj == rows_all[r0:r1, None], np.inf, d2r)
        order = np.lexsort((cjc, d2m), axis=-1)
        sel = order[:, :k]
        selj[r0:r1] = np.take_along_axis(cj, sel, 1)
        seld2[r0:r1] = np.take_along_axis(d2m, sel, 1)
        part = np.partition(d2r, k, axis=1)
        cutoff[r0:r1] = part[:, k]

    # Safety: if the 24th subchunk max is at/above the (k+1)-th best
    # candidate (within fp16-quantization + matmul-error margin), unselected
    # subchunks could hide a true neighbor -> exact rescan of those rows.
    v24 = vals_all[:, NSEL - 1].astype(np.float32)
    margin = 0.02 + np.abs(v24) * 2.0**-9
    flagged = (-v24) <= cutoff + margin
    flagged |= ~np.isfinite(seld2[:, -1])

    for i in np.nonzero(flagged)[0]:
        d2i = x2[i] + x2 - 2.0 * (pos32 @ pos32[i])
        d2i[i] = np.inf
        oi = np.lexsort((np.arange(n), d2i))[:k]
        selj[i] = oi
        seld2[i] = d2i[oi]

    dist = np.sqrt(np.maximum(seld2, 0.0)).astype(np.float32)
    dst = np.repeat(np.arange(n, dtype=np.int32), k)
    edge_index = np.stack([selj.reshape(-1).astype(np.int32), dst])
    return edge_index, dist.reshape(-1)


def _postprocess(pos32, cand_all, k):
    """cand_all [N, NCHUNK*8] uint16 chunk-local indices -> (edge_index, dist)."""
    n = pos32.shape[0]
    x2 = np.sum(pos32 * pos32, axis=-1)  # f32, same op order as reference

    base = (np.arange(NCHUNK * 8, dtype=np.int32) // 8) * CHUNK
    gj = cand_all.astype(np.int32) + base[None, :]          # global ids
    valid = gj < n
    gjc = np.minimum(gj, n - 1)

    pj = pos32[gjc]                                          # [n, 80, 3]
    dot = np.einsum("nd,ncd->nc", pos32, pj)
    d2r = x2[:, None] + x2[gjc] - 2.0 * dot                  # rescored, f32
    d2r = np.where(valid, d2r, np.inf).astype(np.float32)

    rows = np.arange(n, dtype=np.int32)[:, None]
    d2m = np.where(gj == rows, np.inf, d2r)                  # self excluded

    order = np.lexsort((gjc, d2m), axis=-1)
    sel = order[:, :k]
    selj = np.take_along_axis(gj, sel, 1)
    seld2 = np.take_along_axis(d2m, sel, 1)

    # --- safety: rows where a saturated chunk could hide a missed neighbor
    part = np.partition(d2r, k, axis=1)                      # self included
    cutoff = part[:, k]                                      # (k+1)-th smallest
    chunk_max = d2r.reshape(n, NCHUNK, 8).max(axis=2)
    flagged = np.any(chunk_max <= cutoff[:, None] + MARGIN, axis=1)
    flagged |= ~np.isfinite(seld2[:, -1])

    for i in np.nonzero(flagged)[0]:
        d2i = x2[i] + x2 - 2.0 * (pos32 @ pos32[i])
        d2i[i] = np.inf
        oi = np.lexsort((np.arange(n), d2i))[:k]
        selj[i] = oi
        seld2[i] = d2i[oi]

    dist = np.sqrt(np.maximum(seld2, 0.0)).astype(np.float32)
    dst = np.repeat(np.arange(n, dtype=np.int32), k)
    edge_index = np.stack([selj.reshape(-1).astype(np.int32), dst])
    return edge_index, dist.reshape(-1)


def kernel(pos, k):
    global _last_exec_time_ns, _last_results
    k = int(k)
    pos32 = np.ascontiguousarray(np.asarray(pos), dtype=np.float32)
    assert pos32.shape == (N, 3), pos32.shape

    qT_all, pTf = _prep_operands(pos32)
    x2_64 = np.sum(pos32.astype(np.float64) ** 2, axis=-1)
    nx2 = (-x2_64).astype(np.float32)
    in_maps = []
    for c in range(NCORES):
        nx2q = np.ascontiguousarray(
            nx2[c * R : (c + 1) * R].reshape(R // BLOCK, BLOCK).T
        )
        in_maps.append(
            {
                "qpT": np.ascontiguousarray(
                    np.concatenate([qT_all[:, c * R : (c + 1) * R], pTf], axis=1)
                ),
                "nx2q": nx2q,
            }
        )

    nc = build_kernel_v2()
    res = run_bass_kernel_spmd(nc, in_maps, core_ids=list(range(NCORES)))
    _last_exec_time_ns = res.exec_time_ns
    _last_results = res

    def _decode(arr, w):
        # [block, nblock*w] -> [r, w] (row b*block+p = arr[p, b*w:(b+1)*w])
        a = arr.reshape(BLOCK, R // BLOCK, w)
        return a.transpose(1, 0, 2).reshape(R, w)

    ids_all = np.concatenate(
        [_decode(res.results[c]["ids"], NSEL) for c in range(NCORES)], axis=0
    )
    vals_all = np.concatenate(
        [_decode(res.results[c]["vals"], NSEL) for c in range(NCORES)], axis=0
    )
    return _postprocess_v2(pos32, ids_all, vals_all, k)
